# revision 7
# baseline (speedup 1.0000x reference)
"""Masked self-attention Trainium2 Bass kernel.

Reference computation (per batch b):
    q = x @ Wq + bq ; k = x @ Wk + bk ; v = x @ Wv + bv      # [S, A]
    scores = (q @ k.T) / sqrt(S)  with causal mask            # [S, S]
    out = softmax(scores, axis=-1) @ v                        # [S, A]

Sharding: data-parallel over batch across 8 NeuronCores (B=32 -> 4 per core),
weights replicated. No collectives.

Per-core design. Mixed precision: xT / W / expT / v are bf16; qT / kT are
fp8e4 feeding DoubleRow score matmuls (2 MACs/cell/cycle, contraction 256 of
A per matmul); PSUM accumulation and DRAM-facing input/output stay fp32.
Measured rel err ~1.0e-2 vs the 2e-2 gate (fp8 q/k rounding dominates; the
softmax numerator/denominator share the same rounded weights so most of the
exp error cancels).

The PE never transposes: x [S,E] fp32 is DMA'd to SBUF, converted fp32->bf16
on Pool/DVE, and transposed bf16 SBUF->SBUF by the XBAR DMA-transpose unit
(InstDmaTransposeAnt, out[p,u,c] = in[c, u*128+p]) straight into the xT
layout the projections consume. xT is stored s-tile-major [P, 4, n_e, P] per
s-half so each transpose writes a contiguous [P, n_e*P] slice (XBAR requires
contiguous destinations); projection matmuls read [:, :, u, :] 3D moving APs
spanning the 4 s-tiles. The last s-tile (104 valid rows) rides a bf16 tile
whose tail rows are zeroed once, so xT cols 1000:1024 are clean zeros.

  Stage A: per s-tile: 2-chunk DMA load (sync queue) -> fp32->bf16 copy
           (Pool; DVE/Pool alternating for batch 0 where DVE is idle) ->
           XBAR transpose (sync queue) into xT.
  Stage B: qT/kT = W.T @ xT -> fp8 DoubleRow pair tiles [P, 2, 1024] with
           bias folded into the ACT PSUM->SBUF copy; q/k stay UNSCALED
           (1/sqrt(S) rides the exp's scale). v = xT.T @ Wv -> [S, A+2] bf16
           with bv added on DVE; the last two columns are constant ones
           (written once) that make the PV matmul emit softmax row-sums.
           Order: v(0..3), q-half0, k-half0, v(4..7), q-half1, k-half1 so
           batch 0's PE work starts after a single transposed tile.
  Stage C: scoresT[k,q] = kT.T @ qT per k-tile in causal-trimmed chunks of
           2 DoubleRow matmuls; exp(scale=1/sqrt(S)) on ACT -> bf16 expT;
           the diagonal block's upper triangle is zeroed in expT by a Pool
           affine_select (no PSUM mask add, no masked-scores overflow risk:
           |scores|/sqrt(S) <~ 5). No max-subtraction.
  Stage D: interleaved with C with one tile of score lookahead: out_psum =
           sum_t expT[t].T @ v_aug[t] in two column chunks on double-buffered
           PSUM banks; DVE reciprocal of the ones-column row-sum scales both
           halves; DMA out per 256-column half (sync queue).

Cross-batch software pipelining: x loads for batch b+1 are emitted at the
start of stage B(b) (they fill during B/C/D); the bf16 converts + XBAR
transposes for batch b+1 are interleaved into batch b's C/D emission (one
s-tile per pv iteration) so the in-order Pool/sync streams stay dense and
batch b+1's projections find xT ready. xT double-buffers by batch parity.
"""

import numpy as np
from contextlib import ExitStack

import concourse.bass as bass
import concourse.mybir as mybir
import concourse.tile as tile
from concourse import bacc
from concourse.bass_utils import run_bass_kernel_spmd

P = 128
F32 = mybir.dt.float32
BF16 = mybir.dt.bfloat16
FP8 = mybir.dt.float8e4
DR = mybir.MatmulPerfMode.DoubleRow
AF = mybir.ActivationFunctionType

N_CORES = 8
B, S, E, A = 32, 1000, 1024, 512


def _even_chunks(start, total, maxc):
    """Split [start, start+total) into ceil(total/maxc) near-even chunks,
    each of even size (required by DoubleRow moving dim)."""
    assert total % 2 == 0
    n = max(1, -(-total // maxc))
    bounds = [start + ((i * total) // n) // 2 * 2 for i in range(n)]
    bounds.append(start + total)
    return [(bounds[i], bounds[i + 1] - bounds[i]) for i in range(n)]


def build(b_pc, s, e, a, reps=1, warm_weights=False):
    # warm_weights: skip the DRAM weight loads (timing-only simulation; the
    # measured rep-loop slope never sees the weight-load transient).
    assert e % P == 0 and a % P == 0
    n_s = -(-s // P)
    n_e = e // P
    n_a = a // P
    assert n_s == 8 and n_e == 8 and n_a == 4
    inv_den = float(s) ** -0.5
    s_tiles = [(t * P, min(P, s - t * P)) for t in range(n_s)]
    s_pad = n_s * P  # 1024; cols s..s_pad are zeros (zero-tail bf16 tile)
    h = a // 2  # PV column split: [0,h) and [h, a+2)
    nt_h = n_s // 2  # s-tiles per half

    nc = bacc.Bacc("TRN2")
    x = nc.dram_tensor("x", [b_pc, s, e], F32, kind="ExternalInput").ap()
    w_dram = {
        "q": nc.dram_tensor("Wq", [e, a], F32, kind="ExternalInput").ap(),
        "k": nc.dram_tensor("Wk", [e, a], F32, kind="ExternalInput").ap(),
        "v": nc.dram_tensor("Wv", [e, a], F32, kind="ExternalInput").ap(),
    }
    b_dram = {
        "q": nc.dram_tensor("bq", [a], F32, kind="ExternalInput").ap(),
        "k": nc.dram_tensor("bk", [a], F32, kind="ExternalInput").ap(),
        "v": nc.dram_tensor("bv", [a], F32, kind="ExternalInput").ap(),
    }
    out = nc.dram_tensor("out", [b_pc, s, a], F32, kind="ExternalOutput").ap()

    with tile.TileContext(nc) as tc, ExitStack() as ctx:
        pool = ctx.enter_context(tc.tile_pool(name="sb", bufs=1))
        pp_proj = ctx.enter_context(tc.tile_pool(name="pp_proj", bufs=2, space="PSUM"))
        pp_score = ctx.enter_context(tc.tile_pool(name="pp_sc", bufs=2, space="PSUM"))
        pp_o1 = ctx.enter_context(tc.tile_pool(name="pp_o1", bufs=2, space="PSUM"))
        pp_o2 = ctx.enter_context(tc.tile_pool(name="pp_o2", bufs=2, space="PSUM"))

        # ---------------- weights / biases ----------------
        w_sb = {}
        warm_engs = [nc.vector, nc.gpsimd]
        for wi, nm in enumerate(("q", "k", "v")):
            tiles = []
            for u in range(n_e):
                w_r = pool.tile([P, a], BF16, tag=f"w_{nm}", bufs=n_e)
                if warm_weights:
                    # timing-only simulation: weights counted as resident
                    # (memsets spread across engines so no single engine's
                    # stream delays batch 0; first-exec-only transient anyway)
                    warm_engs[(wi * n_e + u) % 2].memset(w_r, 0.0)
                    tiles.append(w_r)
                    continue
                w_stage = pool.tile([P, a], F32, tag="w_stage", bufs=2)
                nc.gpsimd.dma_start(
                    w_stage[:], w_dram[nm][u * P:(u + 1) * P, :])
                nc.vector.tensor_copy(w_r[:], w_stage[:])
                tiles.append(w_r)
            w_sb[nm] = tiles

        bias_sb = {}
        for nm in ("q", "k"):
            b_st = pool.tile([P, n_a], F32, tag=f"b_{nm}", bufs=1)
            nc.gpsimd.dma_start(
                b_st[:], b_dram[nm].rearrange("(m p) -> p m", p=P)
            )
            bias_sb[nm] = b_st
        bv_stage = pool.tile([1, a], F32)
        nc.gpsimd.dma_start(bv_stage[:], b_dram["v"][:])
        bv_bc = pool.tile([P, a], F32)
        nc.gpsimd.partition_broadcast(bv_bc[:], bv_stage[:])

        # ---------------- persistent per-tile SBUF slots ----------------
        x_slots = [pool.tile([P, e], F32, tag=f"x{t}", bufs=1, name=f"x{t}")
                   for t in range(n_s)]
        xbf = [pool.tile([P, e], BF16, tag=f"xbf{t}", bufs=1, name=f"xbf{t}")
               for t in range(n_s)]
        # zero the last tile's invalid rows once: transposes always read the
        # full 128 rows, so xT cols s..s_pad become clean zeros.
        if s_tiles[-1][1] < P:
            nc.gpsimd.memset(xbf[-1][:], 0.0)
        # xT[par][hi]: s-tile-major [P, nt_h, n_e, P]; par = batch parity
        xT = [
            [pool.tile([P, nt_h, n_e, P], BF16, tag=f"xT{par}_{hi}", bufs=1,
                        name=f"xT{par}_{hi}")
             for hi in range(2)]
            for par in range(2)
        ]
        # v with two constant ones-columns (softmax row-sums via PV matmul)
        v_slots = [pool.tile([P, a + 2], BF16, tag=f"v{t}", bufs=1, name=f"v{t}")
                   for t in range(n_s)]
        for t in range(n_s):
            nc.gpsimd.memset(v_slots[t][:, a:a + 2], 1.0)
        # qT/kT fp8 DoubleRow pair tiles: a-tiles (2m, 2m+1) on the pair dim
        qkT = {
            nm: [pool.tile([P, 2, s_pad], FP8, tag=f"{nm}P{m2}", bufs=1,
                           name=f"{nm}P{m2}")
                 for m2 in range(n_a // 2)]
            for nm in ("q", "k")
        }
        expT = [pool.tile([P, s - k0], BF16, tag=f"expT{t}", bufs=1,
                          name=f"expT{t}")
                for t, (k0, _) in enumerate(s_tiles)]

        # ---------------- emission helpers ----------------
        def emit_load(b, t):
            s0, sl = s_tiles[t]
            wsp = e // 2
            for qi in range(2):
                nc.sync.dma_start(
                    x_slots[t][:sl, qi * wsp:(qi + 1) * wsp],
                    x[b, s0:s0 + sl, qi * wsp:(qi + 1) * wsp],
                )

        def emit_conv_xbar(t, eng=None):
            sl = s_tiles[t][1]
            (eng or nc.gpsimd).tensor_copy(xbf[t][:sl, :], x_slots[t][:sl, :])
            return t

        def emit_xbar(b, t):
            par = b % 2
            nc.sync.dma_start_transpose(
                xT[par][t // nt_h][:, t % nt_h, :, :], xbf[t][:])

        def proj_chunk(b, nm, hi):
            par = b % 2
            c0 = hi * (nt_h * P)
            for m in range(n_a):
                mm = pp_proj.tile([P, 512], F32, tag="proj")
                for u in range(n_e):
                    nc.tensor.matmul(
                        mm[:],
                        w_sb[nm][u][:, m * P:(m + 1) * P],
                        xT[par][hi][:, :, u, :],
                        start=(u == 0), stop=(u == n_e - 1),
                    )
                nc.scalar.activation(
                    qkT[nm][m // 2][:, m % 2, c0:c0 + nt_h * P], mm[:],
                    AF.Identity, bias=bias_sb[nm][:, m:m + 1],
                )

        def v_tile(b, t):
            par = b % 2
            s0, sl = s_tiles[t]
            vm = pp_proj.tile([P, 512], F32, tag="proj")
            for u in range(n_e):
                nc.tensor.matmul(
                    vm[:sl, :a],
                    xT[par][t // nt_h][:, t % nt_h, u, :sl],
                    w_sb["v"][u][:],
                    start=(u == 0), stop=(u == n_e - 1),
                )
            nc.vector.tensor_add(
                v_slots[t][:sl, :a], vm[:sl, :a], bv_bc[:sl, :])

        def scores_tile(t):
            k0, kl = s_tiles[t]
            et = expT[t]
            for pi, (c0, cl) in enumerate(_even_chunks(k0, s - k0, 512)):
                sc = pp_score.tile([P, 512], F32, tag="score")
                for m2 in range(n_a // 2):
                    nc.tensor.matmul(
                        sc[:kl, :cl],
                        qkT["k"][m2][:, :, k0:k0 + kl],
                        qkT["q"][m2][:, :, c0:c0 + cl],
                        start=(m2 == 0), stop=(m2 == n_a // 2 - 1),
                        perf_mode=DR,
                    )
                nc.scalar.activation(
                    et[:kl, c0 - k0:c0 - k0 + cl], sc[:kl, :cl], AF.Exp,
                    scale=inv_den,
                )
                if pi == 0:
                    # zero the upper triangle (q < k) of the diagonal block
                    nc.gpsimd.affine_select(
                        out=et[:kl, :kl], in_=et[:kl, :kl],
                        compare_op=mybir.AluOpType.is_ge,
                        fill=0.0, base=0,
                        pattern=[[1, kl]], channel_multiplier=-1,
                    )

        def pv_tile(b, i):
            q0, il = s_tiles[i]
            op1 = pp_o1.tile([P, h], F32, tag="op1")
            op2 = pp_o2.tile([P, a - h + 2], F32, tag="op2")
            for t in range(i + 1):
                k0t, klt = s_tiles[t]
                lhs = expT[t][:klt, q0 - k0t:q0 - k0t + il]
                nc.tensor.matmul(
                    op1[:il, :], lhs, v_slots[t][:klt, 0:h],
                    start=(t == 0), stop=(t == i),
                )
                nc.tensor.matmul(
                    op2[:il, :], lhs, v_slots[t][:klt, h:a + 2],
                    start=(t == 0), stop=(t == i),
                )
            rec = pool.tile([P, 1], F32, tag="rec", bufs=2, name="rec")
            nc.vector.reciprocal(rec[:il, :], op2[:il, a - h:a - h + 1])
            o_sb = pool.tile([P, a], F32, tag="o_sb", bufs=3, name="o_sb")
            nc.vector.tensor_scalar_mul(
                o_sb[:il, 0:h], op1[:il, :], rec[:il, 0:1])
            nc.sync.dma_start(out[b, q0:q0 + il, 0:h], o_sb[:il, 0:h])
            nc.vector.tensor_scalar_mul(
                o_sb[:il, h:a], op2[:il, 0:a - h], rec[:il, 0:1])
            nc.sync.dma_start(out[b, q0:q0 + il, h:a], o_sb[:il, h:a])

        # ---------------- per-batch pipeline ----------------
        rep_ctx = (tc.For_i(0, reps, 1, hint_engines=tuple(nc.engines),
                            staggered_reset=True)
                   if reps > 1 else None)
        if rep_ctx is not None:
            ctx.enter_context(rep_ctx)

        for b in range(b_pc):
            if b == 0:
                # cold stage A for batch 0: load/convert/transpose chain,
                # convert engine alternating DVE/Pool (DVE idle at rep start)
                for t in range(n_s):
                    emit_load(0, t)
                    emit_conv_xbar(t, nc.vector if t % 2 == 0 else nc.gpsimd)
                    emit_xbar(0, t)
            # prefetch next batch's x during stage B/C/D of this one
            if b + 1 < b_pc:
                for t in range(n_s):
                    emit_load(b + 1, t)
            # ---- stage B ----
            for t in range(0, nt_h):
                v_tile(b, t)
            proj_chunk(b, "q", 0)
            proj_chunk(b, "k", 0)
            for t in range(nt_h, n_s):
                v_tile(b, t)
            proj_chunk(b, "q", 1)
            proj_chunk(b, "k", 1)
            # ---- stage C/D with one tile of score lookahead; batch b+1's
            # convert+transpose interleaved one s-tile per pv iteration ----
            scores_tile(0)
            for i in range(n_s):
                if i + 1 < n_s:
                    scores_tile(i + 1)
                if b + 1 < b_pc:
                    emit_conv_xbar(i)
                    emit_xbar(b + 1, i)
                pv_tile(b, i)

    nc.compile()
    return nc


_BUILT = {}


def _get_nc(b_pc, s, e, a):
    key = (b_pc, s, e, a)
    if key not in _BUILT:
        _BUILT[key] = build(b_pc, s, e, a)
    return _BUILT[key]


def run_sharded(inputs, b_pc, s, e, a, **run_kwargs):
    """Run the SPMD kernel over N_CORES cores, sharding batch dim of x."""
    x = np.ascontiguousarray(inputs["x"], dtype=np.float32)
    b_total = x.shape[0]
    assert b_total == b_pc * N_CORES
    shared = {
        "Wq": np.ascontiguousarray(inputs["Wq"], dtype=np.float32),
        "Wk": np.ascontiguousarray(inputs["Wk"], dtype=np.float32),
        "Wv": np.ascontiguousarray(inputs["Wv"], dtype=np.float32),
        "bq": np.ascontiguousarray(inputs["bq"], dtype=np.float32),
        "bk": np.ascontiguousarray(inputs["bk"], dtype=np.float32),
        "bv": np.ascontiguousarray(inputs["bv"], dtype=np.float32),
    }
    in_maps = [
        {"x": x[c * b_pc:(c + 1) * b_pc], **shared} for c in range(N_CORES)
    ]
    nc = _get_nc(b_pc, s, e, a)
    res = run_bass_kernel_spmd(nc, in_maps, core_ids=list(range(N_CORES)),
                               **run_kwargs)
    full = np.concatenate([res.results[c]["out"] for c in range(N_CORES)], axis=0)
    return full, res


def kernel(**inputs) -> np.ndarray:
    out, _ = run_sharded(inputs, B // N_CORES, S, E, A)
    return out


# revision 35
# speedup vs baseline: 1.3128x; 1.3128x over previous
"""Masked self-attention Trainium2 Bass kernel.

Reference computation (per batch b):
    q = x @ Wq + bq ; k = x @ Wk + bk ; v = x @ Wv + bv      # [S, A]
    scores = (q @ k.T) / sqrt(S)  with causal mask            # [S, S]
    out = softmax(scores, axis=-1) @ v                        # [S, A]

Sharding: data-parallel over batch across 8 NeuronCores (B=32 -> 4 per core),
weights replicated. No collectives.

Per-core design. Mixed precision: xT / W / expT / v are bf16; qT / kT are
fp8e4 feeding DoubleRow score matmuls (2 MACs/cell/cycle, contraction 256 of
A per matmul); PSUM accumulation and DRAM-facing input/output stay fp32.
Measured rel err ~1.0e-2 vs the 2e-2 gate (fp8 q/k rounding dominates; the
softmax numerator/denominator share the same rounded weights so most of the
exp error cancels).

The PE never transposes: x [S,E] fp32 is DMA'd to SBUF, converted fp32->bf16
on Pool/DVE, and transposed bf16 SBUF->SBUF by the XBAR DMA-transpose unit
(InstDmaTransposeAnt, out[p,u,c] = in[c, u*128+p]) straight into the xT
layout the projections consume. xT is stored s-tile-major [P, 4, n_e, P] per
s-half so each transpose writes a contiguous [P, n_e*P] slice (XBAR requires
contiguous destinations); projection matmuls read [:, :, u, :] 3D moving APs
spanning the 4 s-tiles. The last s-tile (104 valid rows) rides a bf16 tile
whose tail rows are zeroed once, so xT cols 1000:1024 are clean zeros.

  Stage A: per s-tile: 2-chunk DMA load (sync queue) -> fp32->bf16 copy
           (Pool; DVE/Pool alternating for batch 0 where DVE is idle) ->
           XBAR transpose (sync queue) into xT.
  Stage B: qT/kT = W.T @ xT -> fp8 DoubleRow pair tiles [P, 2, 1024] with
           bias folded into the ACT PSUM->SBUF copy; q/k stay UNSCALED
           (1/sqrt(S) rides the exp's scale). v = xT.T @ Wv -> [S, A+2] bf16
           with bv added on DVE; the last two columns are constant ones
           (written once) that make the PV matmul emit softmax row-sums.
           Order: v(0..3), q-half0, k-half0, v(4..7), q-half1, k-half1 so
           batch 0's PE work starts after a single transposed tile.
  Stage C: scoresT[k,q] = kT.T @ qT per k-tile in causal-trimmed chunks of
           2 DoubleRow matmuls; exp(scale=1/sqrt(S)) on ACT -> bf16 expT;
           the diagonal block's upper triangle is zeroed in expT by a Pool
           affine_select (no PSUM mask add, no masked-scores overflow risk:
           |scores|/sqrt(S) <~ 5). No max-subtraction.
  Stage D: interleaved with C with one tile of score lookahead: out_psum =
           sum_t expT[t].T @ v_aug[t] in two column chunks on double-buffered
           PSUM banks; DVE reciprocal of the ones-column row-sum scales both
           halves; DMA out per 256-column half (sync queue).

Cross-batch software pipelining: x loads for batch b+1 are emitted at the
start of stage B(b) (they fill during B/C/D); the bf16 converts + XBAR
transposes for batch b+1 are interleaved into batch b's C/D emission (one
s-tile per pv iteration) so the in-order Pool/sync streams stay dense and
batch b+1's projections find xT ready. xT double-buffers by batch parity.
"""

import numpy as np
from contextlib import ExitStack

import concourse.bass as bass
import concourse.mybir as mybir
import concourse.tile as tile
from concourse import bacc
from concourse.bass_utils import run_bass_kernel_spmd

P = 128
F32 = mybir.dt.float32
BF16 = mybir.dt.bfloat16
FP8 = mybir.dt.float8e4
DR = mybir.MatmulPerfMode.DoubleRow
AF = mybir.ActivationFunctionType

N_CORES = 8
B, S, E, A = 32, 1000, 1024, 512


def _even_chunks(start, total, maxc):
    """Split [start, start+total) into ceil(total/maxc) near-even chunks,
    each of even size (required by DoubleRow moving dim)."""
    assert total % 2 == 0
    n = max(1, -(-total // maxc))
    bounds = [start + ((i * total) // n) // 2 * 2 for i in range(n)]
    bounds.append(start + total)
    return [(bounds[i], bounds[i + 1] - bounds[i]) for i in range(n)]


def build(b_pc, s, e, a, reps=1, warm_weights=False):
    # warm_weights: skip the DRAM weight loads (timing-only simulation; the
    # measured rep-loop slope never sees the weight-load transient).
    assert e % P == 0 and a % P == 0
    n_s = -(-s // P)
    n_e = e // P
    n_a = a // P
    assert n_s == 8 and n_e == 8 and n_a == 4
    inv_den = float(s) ** -0.5
    s_tiles = [(t * P, min(P, s - t * P)) for t in range(n_s)]
    s_pad = n_s * P  # 1024; cols s..s_pad are zeros (zero-tail bf16 tile)
    h = a // 2  # PV column split: [0,h) and [h, a+2)
    nt_h = n_s // 2  # s-tiles per half

    nc = bacc.Bacc("TRN2")
    x = nc.dram_tensor("x", [b_pc, s, e], F32, kind="ExternalInput").ap()
    w_dram = {
        "q": nc.dram_tensor("Wq", [e, a], F32, kind="ExternalInput").ap(),
        "k": nc.dram_tensor("Wk", [e, a], F32, kind="ExternalInput").ap(),
        "v": nc.dram_tensor("Wv", [e, a], F32, kind="ExternalInput").ap(),
    }
    b_dram = {
        "q": nc.dram_tensor("bq", [a], F32, kind="ExternalInput").ap(),
        "k": nc.dram_tensor("bk", [a], F32, kind="ExternalInput").ap(),
        "v": nc.dram_tensor("bv", [a], F32, kind="ExternalInput").ap(),
    }
    out = nc.dram_tensor("out", [b_pc, s, a], F32, kind="ExternalOutput").ap()

    with tile.TileContext(nc) as tc, ExitStack() as ctx:
        pool = ctx.enter_context(tc.tile_pool(name="sb", bufs=1))
        pp_proj = ctx.enter_context(tc.tile_pool(name="pp_proj", bufs=2, space="PSUM"))
        pp_score = ctx.enter_context(tc.tile_pool(name="pp_sc", bufs=2, space="PSUM"))
        pp_o1 = ctx.enter_context(tc.tile_pool(name="pp_o1", bufs=2, space="PSUM"))
        pp_o2 = ctx.enter_context(tc.tile_pool(name="pp_o2", bufs=2, space="PSUM"))

        # ---------------- weights / biases ----------------
        w_sb = {}
        warm_engs = [nc.vector, nc.gpsimd]
        for wi, nm in enumerate(("q", "k", "v")):
            tiles = []
            for u in range(n_e):
                w_r = pool.tile([P, a], BF16, tag=f"w_{nm}", bufs=n_e)
                if warm_weights:
                    # timing-only simulation: weights counted as resident
                    # (memsets spread across engines so no single engine's
                    # stream delays batch 0; first-exec-only transient anyway)
                    warm_engs[(wi * n_e + u) % 2].memset(w_r, 0.0)
                    tiles.append(w_r)
                    continue
                w_stage = pool.tile([P, a], F32, tag="w_stage", bufs=2)
                nc.gpsimd.dma_start(
                    w_stage[:], w_dram[nm][u * P:(u + 1) * P, :])
                nc.vector.tensor_copy(w_r[:], w_stage[:])
                tiles.append(w_r)
            w_sb[nm] = tiles

        bias_sb = {}
        for nm in ("q", "k"):
            b_st = pool.tile([P, n_a], F32, tag=f"b_{nm}", bufs=1)
            nc.gpsimd.dma_start(
                b_st[:], b_dram[nm].rearrange("(m p) -> p m", p=P)
            )
            bias_sb[nm] = b_st
        bv_stage = pool.tile([1, a], F32)
        nc.gpsimd.dma_start(bv_stage[:], b_dram["v"][:])
        bv_bc = pool.tile([P, a], F32)
        nc.gpsimd.partition_broadcast(bv_bc[:], bv_stage[:])

        # additive causal mask for the diagonal block:
        # keep 0 where col q >= row k, else fill -1e9
        amask = pool.tile([P, P], F32)
        nc.gpsimd.memset(amask, 0.0)
        nc.gpsimd.affine_select(
            out=amask, in_=amask,
            compare_op=mybir.AluOpType.is_ge,
            fill=-1.0e9, base=0,
            pattern=[[1, P]], channel_multiplier=-1,
        )



        # ---------------- persistent per-tile SBUF slots ----------------
        x_slots = [pool.tile([P, e], F32, tag=f"x{t}", bufs=1, name=f"x{t}")
                   for t in range(n_s)]
        xbf = [pool.tile([P, e], BF16, tag=f"xbf{t}", bufs=1, name=f"xbf{t}")
               for t in range(n_s)]
        # zero the last tile's invalid rows once: transposes always read the
        # full 128 rows, so xT cols s..s_pad become clean zeros.
        if s_tiles[-1][1] < P:
            nc.gpsimd.memset(xbf[-1][:], 0.0)
        # xT[par][hi]: s-tile-major [P, nt_h, n_e, P]; par = batch parity
        xT = [
            [pool.tile([P, nt_h, n_e, P], BF16, tag=f"xT{par}_{hi}", bufs=1,
                        name=f"xT{par}_{hi}")
             for hi in range(2)]
            for par in range(2)
        ]
        # v with two constant ones-columns (softmax row-sums via PV matmul);
        # double-buffered by batch parity: v(b+1) is computed during C/D(b),
        # which still reads v(b)
        v_slots = [
            [pool.tile([P, a + 2], BF16, tag=f"v{par}_{t}", bufs=1,
                       name=f"v{par}_{t}")
             for t in range(n_s)]
            for par in range(2)
        ]
        for par in range(2):
            for t in range(n_s):
                nc.gpsimd.memset(v_slots[par][t][:, a:a + 2], 1.0)
        # qT/kT fp8 DoubleRow pair tiles: a-tiles (2m, 2m+1) on the pair dim
        qkT = {
            nm: [pool.tile([P, 2, s_pad], FP8, tag=f"{nm}P{m2}", bufs=1,
                           name=f"{nm}P{m2}")
                 for m2 in range(n_a // 2)]
            for nm in ("q", "k")
        }
        expT = [pool.tile([P, s - k0], BF16, tag=f"expT{t}", bufs=1,
                          name=f"expT{t}")
                for t, (k0, _) in enumerate(s_tiles)]

        # ---------------- emission helpers ----------------
        def emit_load(b, t):
            s0, sl = s_tiles[t]
            wsp = e // 2
            for qi in range(2):
                nc.sync.dma_start(
                    x_slots[t][:sl, qi * wsp:(qi + 1) * wsp],
                    x[b, s0:s0 + sl, qi * wsp:(qi + 1) * wsp],
                )

        def emit_conv_xbar(t, eng=None):
            sl = s_tiles[t][1]
            (eng or nc.gpsimd).tensor_copy(xbf[t][:sl, :], x_slots[t][:sl, :])
            return t

        def emit_xbar(b, t):
            par = b % 2
            nc.sync.dma_start_transpose(
                xT[par][t // nt_h][:, t % nt_h, :, :], xbf[t][:])

        def proj_chunk(b, nm, hi):
            par = b % 2
            c0 = hi * (nt_h * P)
            for m in range(n_a):
                mm = pp_proj.tile([P, 512], F32, tag="proj")
                for u in range(n_e):
                    nc.tensor.matmul(
                        mm[:],
                        w_sb[nm][u][:, m * P:(m + 1) * P],
                        xT[par][hi][:, :, u, :],
                        start=(u == 0), stop=(u == n_e - 1),
                    )
                nc.scalar.activation(
                    qkT[nm][m // 2][:, m % 2, c0:c0 + nt_h * P], mm[:],
                    AF.Identity, bias=bias_sb[nm][:, m:m + 1],
                )

        def v_tile(b, t):
            par = b % 2
            s0, sl = s_tiles[t]
            vm = pp_proj.tile([P, 512], F32, tag="proj")
            for u in range(n_e):
                nc.tensor.matmul(
                    vm[:sl, :a],
                    xT[par][t // nt_h][:, t % nt_h, u, :sl],
                    w_sb["v"][u][:],
                    start=(u == 0), stop=(u == n_e - 1),
                )
            nc.vector.tensor_add(
                v_slots[par][t][:sl, :a], vm[:sl, :a], bv_bc[:sl, :])

        def scores_tile(t):
            k0, kl = s_tiles[t]
            et = expT[t]
            # the diagonal block is its own small FIRST chunk: pv(t)'s last
            # accumulation pair needs exactly this chunk of expT, so its
            # mask->exp chain must resolve as early as possible
            chunks = [(k0, kl)]
            if k0 + kl < s:
                chunks += _even_chunks(k0 + kl, s - k0 - kl, 512)
            for pi, (c0, cl) in enumerate(chunks):
                sc = pp_score.tile([P, 512], F32, tag="score")
                for m2 in range(n_a // 2):
                    nc.tensor.matmul(
                        sc[:kl, :cl],
                        qkT["k"][m2][:, :, k0:k0 + kl],
                        qkT["q"][m2][:, :, c0:c0 + cl],
                        start=(m2 == 0), stop=(m2 == n_a // 2 - 1),
                        perf_mode=DR,
                    )
                if pi == 0:
                    # diagonal block: additive causal mask in PSUM (DVE)
                    nc.vector.tensor_add(
                        sc[:kl, :kl], sc[:kl, :kl], amask[:kl, :kl]
                    )
                nc.scalar.activation(
                    et[:kl, c0 - k0:c0 - k0 + cl], sc[:kl, :cl], AF.Exp,
                    scale=inv_den,
                )

        def pv_tile(b, i):
            q0, il = s_tiles[i]
            op1 = pp_o1.tile([P, h], F32, tag="op1")
            op2 = pp_o2.tile([P, a - h + 2], F32, tag="op2")
            par = b % 2
            for t in range(i + 1):
                k0t, klt = s_tiles[t]
                lhs = expT[t][:klt, q0 - k0t:q0 - k0t + il]
                nc.tensor.matmul(
                    op1[:il, :], lhs, v_slots[par][t][:klt, 0:h],
                    start=(t == 0), stop=(t == i),
                )
                nc.tensor.matmul(
                    op2[:il, :], lhs, v_slots[par][t][:klt, h:a + 2],
                    start=(t == 0), stop=(t == i),
                )
            rec = pool.tile([P, 1], F32, tag="rec", bufs=4, name="rec")
            nc.vector.reciprocal(rec[:il, :], op2[:il, a - h:a - h + 1])
            # deep ring: out DMAs may lag several tiles behind the epilogue
            # (DMA_ENGINES serializes them behind prefetched stage-A traffic)
            o_sb = pool.tile([P, a], F32, tag="o_sb", bufs=8, name="o_sb")
            # epilogue halves on different engines (ACT | DVE) so the
            # op1/op2 PSUM banks drain fast
            # both epilogue halves on DVE: ACT stays free for the exp chain,
            # which paces pv. outs ride the scalar HWDGE queue: everything
            # there is PE-gated, so a hoisted future-batch transpose on the
            # sync queue can never head-block the o_sb ring drain
            nc.vector.tensor_scalar_mul(
                o_sb[:il, 0:h], op1[:il, :], rec[:il, 0:1])
            nc.scalar.dma_start(out[b, q0:q0 + il, 0:h], o_sb[:il, 0:h])
            nc.vector.tensor_scalar_mul(
                o_sb[:il, h:a], op2[:il, 0:a - h], rec[:il, 0:1])
            nc.scalar.dma_start(out[b, q0:q0 + il, h:a], o_sb[:il, h:a])

        # ---------------- per-batch pipeline ----------------
        # prologue: cold stage A for batch 0 plus its v tiles (primes the
        # rotated loop body, which computes v(b+1) during C/D(b)). All loads
        # first (the sync queue pipelines them at full rate), then
        # convert+transpose per tile, converts alternating DVE/Pool.
        for t in range(n_s):
            emit_load(0, t)
        for t in range(n_s):
            emit_conv_xbar(t, nc.vector if t % 2 == 0 else nc.gpsimd)
            emit_xbar(0, t)
            v_tile(0, t)

        rep_ctx = (tc.For_i(0, reps, 1, hint_engines=tuple(nc.engines),
                            staggered_reset=True)
                   if reps > 1 else None)
        if rep_ctx is not None:
            ctx.enter_context(rep_ctx)

        for b in range(b_pc):
            # scheduling tier: forbid the scheduler from hoisting batch b+1's
            # engine work (v-adds, bias copies, epilogues) into batch b's
            # streams — cross-batch hoists invert priorities on the in-order
            # engines. Work for batch b+1 emitted in section b (stage-A
            # prefetch) intentionally shares tier b.
            ctx_b = tc.tile_wait_until(b)
            ctx_b.__enter__()
            # every section preps the NEXT batch's x: (b+1)%b_pc — section
            # b_pc-1 preps batch 0 of the next rep-loop iteration, so the
            # loop back-edge barrier costs no refill (software pipelining
            # across the For_i back edge; the prologue primes iteration 1)
            for t in range(n_s):
                emit_load((b + 1) % b_pc, t)
            # ---- stage B (pure projections), with the next batch's
            # convert+transpose interleaved (xbf slots are free: batch b's
            # transposes ran during B(b-1); Pool/DMA are idle during B) ----
            for t in range(n_s):
                emit_conv_xbar(t)
                emit_xbar(b + 1, t)
            proj_chunk(b, "q", 0)
            proj_chunk(b, "k", 0)
            proj_chunk(b, "q", 1)
            proj_chunk(b, "k", 1)
            # ---- stage C/D with one tile of score lookahead, interleaved
            # with the NEXT batch's v tiles (their xT landed during B(b)):
            # ~1.7us of independent PE work per pv iteration covers the
            # cross-engine mask->exp->pv latency chains ----
            scores_tile(0)
            for i in range(n_s):
                if i + 1 < n_s:
                    scores_tile(i + 1)
                v_tile(b + 1, i)
                pv_tile(b, i)
            ctx_b.__exit__(None, None, None)

    nc.compile()
    return nc


_BUILT = {}


def _get_nc(b_pc, s, e, a):
    key = (b_pc, s, e, a)
    if key not in _BUILT:
        _BUILT[key] = build(b_pc, s, e, a)
    return _BUILT[key]


def run_sharded(inputs, b_pc, s, e, a, **run_kwargs):
    """Run the SPMD kernel over N_CORES cores, sharding batch dim of x."""
    x = np.ascontiguousarray(inputs["x"], dtype=np.float32)
    b_total = x.shape[0]
    assert b_total == b_pc * N_CORES
    shared = {
        "Wq": np.ascontiguousarray(inputs["Wq"], dtype=np.float32),
        "Wk": np.ascontiguousarray(inputs["Wk"], dtype=np.float32),
        "Wv": np.ascontiguousarray(inputs["Wv"], dtype=np.float32),
        "bq": np.ascontiguousarray(inputs["bq"], dtype=np.float32),
        "bk": np.ascontiguousarray(inputs["bk"], dtype=np.float32),
        "bv": np.ascontiguousarray(inputs["bv"], dtype=np.float32),
    }
    in_maps = [
        {"x": x[c * b_pc:(c + 1) * b_pc], **shared} for c in range(N_CORES)
    ]
    nc = _get_nc(b_pc, s, e, a)
    res = run_bass_kernel_spmd(nc, in_maps, core_ids=list(range(N_CORES)),
                               **run_kwargs)
    full = np.concatenate([res.results[c]["out"] for c in range(N_CORES)], axis=0)
    return full, res


def kernel(**inputs) -> np.ndarray:
    out, _ = run_sharded(inputs, B // N_CORES, S, E, A)
    return out


# revision 37
# speedup vs baseline: 1.4103x; 1.0743x over previous
"""Masked self-attention Trainium2 Bass kernel.

Reference computation (per batch b):
    q = x @ Wq + bq ; k = x @ Wk + bk ; v = x @ Wv + bv      # [S, A]
    scores = (q @ k.T) / sqrt(S)  with causal mask            # [S, S]
    out = softmax(scores, axis=-1) @ v                        # [S, A]

Sharding: data-parallel over batch across 8 NeuronCores (B=32 -> 4 per core),
weights replicated. No collectives.

Per-core design. Mixed precision: xT / W / expT / v are bf16; qT / kT are
fp8e4 feeding DoubleRow score matmuls (2 MACs/cell/cycle, contraction 256 of
A per matmul); PSUM accumulation and DRAM-facing input/output stay fp32.
Measured rel err ~1.0e-2 vs the 2e-2 gate (fp8 q/k rounding dominates; the
softmax numerator/denominator share the same rounded weights so most of the
exp error cancels).

The PE never transposes: x [S,E] fp32 is DMA'd to SBUF, converted fp32->bf16
on Pool/DVE, and transposed bf16 SBUF->SBUF by the XBAR DMA-transpose unit
(InstDmaTransposeAnt, out[p,u,c] = in[c, u*128+p]) straight into the xT
layout the projections consume. xT is stored s-tile-major [P, 4, n_e, P] per
s-half so each transpose writes a contiguous [P, n_e*P] slice (XBAR requires
contiguous destinations); projection matmuls read [:, :, u, :] 3D moving APs
spanning the 4 s-tiles. The last s-tile (104 valid rows) rides a bf16 tile
whose tail rows are zeroed once, so xT cols 1000:1024 are clean zeros.

  Stage A: per s-tile: 2-chunk DMA load (sync queue) -> fp32->bf16 copy
           (Pool; DVE/Pool alternating for batch 0 where DVE is idle) ->
           XBAR transpose (sync queue) into xT.
  Stage B: qT/kT = W.T @ xT -> fp8 DoubleRow pair tiles [P, 2, 1024] with
           bias folded into the ACT PSUM->SBUF copy; q/k stay UNSCALED
           (1/sqrt(S) rides the exp's scale). v = xT.T @ Wv -> [S, A+2] bf16
           with bv added on DVE; the last two columns are constant ones
           (written once) that make the PV matmul emit softmax row-sums.
           Order: v(0..3), q-half0, k-half0, v(4..7), q-half1, k-half1 so
           batch 0's PE work starts after a single transposed tile.
  Stage C: scoresT[k,q] = kT.T @ qT per k-tile in causal-trimmed chunks of
           2 DoubleRow matmuls; exp(scale=1/sqrt(S)) on ACT -> bf16 expT;
           the diagonal block's upper triangle is zeroed in expT by a Pool
           affine_select (no PSUM mask add, no masked-scores overflow risk:
           |scores|/sqrt(S) <~ 5). No max-subtraction.
  Stage D: interleaved with C with one tile of score lookahead: out_psum =
           sum_t expT[t].T @ v_aug[t] in two column chunks on double-buffered
           PSUM banks; DVE reciprocal of the ones-column row-sum scales both
           halves; DMA out per 256-column half (sync queue).

Cross-batch software pipelining: x loads for batch b+1 are emitted at the
start of stage B(b) (they fill during B/C/D); the bf16 converts + XBAR
transposes for batch b+1 are interleaved into batch b's C/D emission (one
s-tile per pv iteration) so the in-order Pool/sync streams stay dense and
batch b+1's projections find xT ready. xT double-buffers by batch parity.
"""

import numpy as np
from contextlib import ExitStack

import concourse.bass as bass
import concourse.mybir as mybir
import concourse.tile as tile
from concourse import bacc
from concourse.bass_utils import run_bass_kernel_spmd

P = 128
F32 = mybir.dt.float32
BF16 = mybir.dt.bfloat16
FP8 = mybir.dt.float8e4
DR = mybir.MatmulPerfMode.DoubleRow
AF = mybir.ActivationFunctionType

N_CORES = 8
B, S, E, A = 32, 1000, 1024, 512


def _even_chunks(start, total, maxc):
    """Split [start, start+total) into ceil(total/maxc) near-even chunks,
    each of even size (required by DoubleRow moving dim)."""
    assert total % 2 == 0
    n = max(1, -(-total // maxc))
    bounds = [start + ((i * total) // n) // 2 * 2 for i in range(n)]
    bounds.append(start + total)
    return [(bounds[i], bounds[i + 1] - bounds[i]) for i in range(n)]


def build(b_pc, s, e, a, reps=1, warm_weights=False):
    # warm_weights: skip the DRAM weight loads (timing-only simulation; the
    # measured rep-loop slope never sees the weight-load transient).
    assert e % P == 0 and a % P == 0
    n_s = -(-s // P)
    n_e = e // P
    n_a = a // P
    assert n_s == 8 and n_e == 8 and n_a == 4
    inv_den = float(s) ** -0.5
    s_tiles = [(t * P, min(P, s - t * P)) for t in range(n_s)]
    s_pad = n_s * P  # 1024; cols s..s_pad are zeros (zero-tail bf16 tile)
    h = a // 2  # PV column split: [0,h) and [h, a+2)
    nt_h = n_s // 2  # s-tiles per half

    nc = bacc.Bacc("TRN2")
    x = nc.dram_tensor("x", [b_pc, s, e], F32, kind="ExternalInput").ap()
    w_dram = {
        "q": nc.dram_tensor("Wq", [e, a], F32, kind="ExternalInput").ap(),
        "k": nc.dram_tensor("Wk", [e, a], F32, kind="ExternalInput").ap(),
        "v": nc.dram_tensor("Wv", [e, a], F32, kind="ExternalInput").ap(),
    }
    b_dram = {
        "q": nc.dram_tensor("bq", [a], F32, kind="ExternalInput").ap(),
        "k": nc.dram_tensor("bk", [a], F32, kind="ExternalInput").ap(),
        "v": nc.dram_tensor("bv", [a], F32, kind="ExternalInput").ap(),
    }
    out = nc.dram_tensor("out", [b_pc, s, a], F32, kind="ExternalOutput").ap()

    with tile.TileContext(nc) as tc, ExitStack() as ctx:
        pool = ctx.enter_context(tc.tile_pool(name="sb", bufs=1))
        pp_proj = ctx.enter_context(tc.tile_pool(name="pp_proj", bufs=2, space="PSUM"))
        pp_score = ctx.enter_context(tc.tile_pool(name="pp_sc", bufs=2, space="PSUM"))
        pp_o1 = ctx.enter_context(tc.tile_pool(name="pp_o1", bufs=2, space="PSUM"))
        pp_o2 = ctx.enter_context(tc.tile_pool(name="pp_o2", bufs=2, space="PSUM"))

        # ---------------- weights / biases ----------------
        w_sb = {}
        warm_engs = [nc.vector, nc.gpsimd]
        for wi, nm in enumerate(("q", "k", "v")):
            tiles = []
            for u in range(n_e):
                w_r = pool.tile([P, a], BF16, tag=f"w_{nm}", bufs=n_e)
                if warm_weights:
                    # timing-only simulation: weights counted as resident
                    # (memsets spread across engines so no single engine's
                    # stream delays batch 0; first-exec-only transient anyway)
                    warm_engs[(wi * n_e + u) % 2].memset(w_r, 0.0)
                    tiles.append(w_r)
                    continue
                w_stage = pool.tile([P, a], F32, tag="w_stage", bufs=2)
                nc.gpsimd.dma_start(
                    w_stage[:], w_dram[nm][u * P:(u + 1) * P, :])
                nc.vector.tensor_copy(w_r[:], w_stage[:])
                tiles.append(w_r)
            w_sb[nm] = tiles

        bias_sb = {}
        for nm in ("q", "k"):
            b_st = pool.tile([P, n_a], F32, tag=f"b_{nm}", bufs=1)
            nc.gpsimd.dma_start(
                b_st[:], b_dram[nm].rearrange("(m p) -> p m", p=P)
            )
            bias_sb[nm] = b_st
        bv_stage = pool.tile([1, a], F32)
        nc.gpsimd.dma_start(bv_stage[:], b_dram["v"][:])
        bv_bc = pool.tile([P, a], F32)
        nc.gpsimd.partition_broadcast(bv_bc[:], bv_stage[:])

        # additive causal mask for the diagonal block:
        # keep 0 where col q >= row k, else fill -1e9
        amask = pool.tile([P, P], F32)
        nc.gpsimd.memset(amask, 0.0)
        nc.gpsimd.affine_select(
            out=amask, in_=amask,
            compare_op=mybir.AluOpType.is_ge,
            fill=-1.0e9, base=0,
            pattern=[[1, P]], channel_multiplier=-1,
        )



        # ---------------- persistent per-tile SBUF slots ----------------
        x_slots = [pool.tile([P, e], F32, tag=f"x{t}", bufs=1, name=f"x{t}")
                   for t in range(n_s)]
        xbf = [pool.tile([P, e], BF16, tag=f"xbf{t}", bufs=1, name=f"xbf{t}")
               for t in range(n_s)]
        # zero the last tile's invalid rows once: transposes always read the
        # full 128 rows, so xT cols s..s_pad become clean zeros.
        if s_tiles[-1][1] < P:
            nc.gpsimd.memset(xbf[-1][:], 0.0)
        # xT[par][hi]: s-tile-major [P, nt_h, n_e, P]; par = batch parity
        xT = [
            [pool.tile([P, nt_h, n_e, P], BF16, tag=f"xT{par}_{hi}", bufs=1,
                        name=f"xT{par}_{hi}")
             for hi in range(2)]
            for par in range(2)
        ]
        # v with two constant ones-columns (softmax row-sums via PV matmul);
        # double-buffered by batch parity: v(b+1) is computed during C/D(b),
        # which still reads v(b)
        v_slots = [
            [pool.tile([P, a + 2], BF16, tag=f"v{par}_{t}", bufs=1,
                       name=f"v{par}_{t}")
             for t in range(n_s)]
            for par in range(2)
        ]
        for par in range(2):
            for t in range(n_s):
                nc.gpsimd.memset(v_slots[par][t][:, a:a + 2], 1.0)
        # qT/kT fp8 DoubleRow pair tiles: a-tiles (2m, 2m+1) on the pair dim
        qkT = {
            nm: [pool.tile([P, 2, s_pad], FP8, tag=f"{nm}P{m2}", bufs=1,
                           name=f"{nm}P{m2}")
                 for m2 in range(n_a // 2)]
            for nm in ("q", "k")
        }
        expT = [pool.tile([P, s - k0], BF16, tag=f"expT{t}", bufs=1,
                          name=f"expT{t}")
                for t, (k0, _) in enumerate(s_tiles)]

        # ---------------- emission helpers ----------------
        def emit_load(b, t):
            s0, sl = s_tiles[t]
            wsp = e // 2
            for qi in range(2):
                nc.sync.dma_start(
                    x_slots[t][:sl, qi * wsp:(qi + 1) * wsp],
                    x[b, s0:s0 + sl, qi * wsp:(qi + 1) * wsp],
                )

        def emit_conv_xbar(t, eng=None):
            sl = s_tiles[t][1]
            (eng or nc.gpsimd).tensor_copy(xbf[t][:sl, :], x_slots[t][:sl, :])
            return t

        def emit_xbar(b, t):
            par = b % 2
            nc.sync.dma_start_transpose(
                xT[par][t // nt_h][:, t % nt_h, :, :], xbf[t][:])

        def proj_chunk(b, nm, hi):
            par = b % 2
            c0 = hi * (nt_h * P)
            cl = min(nt_h * P, s - c0)  # trim the s..s_pad padding columns
            n_full = cl // P  # full s-tiles in this half
            tail = cl - n_full * P
            for m in range(n_a):
                mm = pp_proj.tile([P, 512], F32, tag="proj")
                for u in range(n_e):
                    nc.tensor.matmul(
                        mm[:, :n_full * P],
                        w_sb[nm][u][:, m * P:(m + 1) * P],
                        xT[par][hi][:, :n_full, u, :],
                        start=(u == 0), stop=(u == n_e - 1),
                    )
                if tail:
                    for u in range(n_e):
                        nc.tensor.matmul(
                            mm[:, n_full * P:cl],
                            w_sb[nm][u][:, m * P:(m + 1) * P],
                            xT[par][hi][:, n_full, u, :tail],
                            start=(u == 0), stop=(u == n_e - 1),
                        )
                nc.scalar.activation(
                    qkT[nm][m // 2][:, m % 2, c0:c0 + cl], mm[:, :cl],
                    AF.Identity, bias=bias_sb[nm][:, m:m + 1],
                )

        def v_tile(b, t):
            par = b % 2
            s0, sl = s_tiles[t]
            vm = pp_proj.tile([P, 512], F32, tag="proj")
            for u in range(n_e):
                nc.tensor.matmul(
                    vm[:sl, :a],
                    xT[par][t // nt_h][:, t % nt_h, u, :sl],
                    w_sb["v"][u][:],
                    start=(u == 0), stop=(u == n_e - 1),
                )
            nc.vector.tensor_add(
                v_slots[par][t][:sl, :a], vm[:sl, :a], bv_bc[:sl, :])

        def scores_tile(t):
            k0, kl = s_tiles[t]
            et = expT[t]
            # the diagonal block is its own small FIRST chunk: pv(t)'s last
            # accumulation pair needs exactly this chunk of expT, so its
            # mask->exp chain must resolve as early as possible
            chunks = [(k0, kl)]
            if k0 + kl < s:
                chunks += _even_chunks(k0 + kl, s - k0 - kl, 512)
            for pi, (c0, cl) in enumerate(chunks):
                sc = pp_score.tile([P, 512], F32, tag="score")
                for m2 in range(n_a // 2):
                    nc.tensor.matmul(
                        sc[:kl, :cl],
                        qkT["k"][m2][:, :, k0:k0 + kl],
                        qkT["q"][m2][:, :, c0:c0 + cl],
                        start=(m2 == 0), stop=(m2 == n_a // 2 - 1),
                        perf_mode=DR,
                    )
                if pi == 0:
                    # diagonal block: additive causal mask in PSUM (DVE)
                    nc.vector.tensor_add(
                        sc[:kl, :kl], sc[:kl, :kl], amask[:kl, :kl]
                    )
                nc.scalar.activation(
                    et[:kl, c0 - k0:c0 - k0 + cl], sc[:kl, :cl], AF.Exp,
                    scale=inv_den,
                )

        def pv_tile(b, i):
            q0, il = s_tiles[i]
            op1 = pp_o1.tile([P, h], F32, tag="op1")
            op2 = pp_o2.tile([P, a - h + 2], F32, tag="op2")
            par = b % 2
            for t in range(i + 1):
                k0t, klt = s_tiles[t]
                lhs = expT[t][:klt, q0 - k0t:q0 - k0t + il]
                nc.tensor.matmul(
                    op1[:il, :], lhs, v_slots[par][t][:klt, 0:h],
                    start=(t == 0), stop=(t == i),
                )
                nc.tensor.matmul(
                    op2[:il, :], lhs, v_slots[par][t][:klt, h:a + 2],
                    start=(t == 0), stop=(t == i),
                )
            rec = pool.tile([P, 1], F32, tag="rec", bufs=4, name="rec")
            nc.vector.reciprocal(rec[:il, :], op2[:il, a - h:a - h + 1])
            # deep ring: out DMAs may lag several tiles behind the epilogue
            # (DMA_ENGINES serializes them behind prefetched stage-A traffic)
            o_sb = pool.tile([P, a], F32, tag="o_sb", bufs=8, name="o_sb")
            # epilogue halves on different engines (ACT | DVE) so the
            # op1/op2 PSUM banks drain fast
            # both epilogue halves on DVE: ACT stays free for the exp chain,
            # which paces pv. outs ride the scalar HWDGE queue: everything
            # there is PE-gated, so a hoisted future-batch transpose on the
            # sync queue can never head-block the o_sb ring drain
            nc.vector.tensor_scalar_mul(
                o_sb[:il, 0:h], op1[:il, :], rec[:il, 0:1])
            nc.scalar.dma_start(out[b, q0:q0 + il, 0:h], o_sb[:il, 0:h])
            nc.vector.tensor_scalar_mul(
                o_sb[:il, h:a], op2[:il, 0:a - h], rec[:il, 0:1])
            nc.scalar.dma_start(out[b, q0:q0 + il, h:a], o_sb[:il, h:a])

        # ---------------- per-batch pipeline ----------------
        # prologue: cold stage A for batch 0 plus its v tiles (primes the
        # rotated loop body, which computes v(b+1) during C/D(b)). All loads
        # first (the sync queue pipelines them at full rate), then
        # convert+transpose per tile, converts alternating DVE/Pool.
        for t in range(n_s):
            emit_load(0, t)
        for t in range(n_s):
            emit_conv_xbar(t, nc.vector if t % 2 == 0 else nc.gpsimd)
            emit_xbar(0, t)
            v_tile(0, t)

        rep_ctx = (tc.For_i(0, reps, 1, hint_engines=tuple(nc.engines),
                            staggered_reset=True)
                   if reps > 1 else None)
        if rep_ctx is not None:
            ctx.enter_context(rep_ctx)

        for b in range(b_pc):
            # scheduling tier: forbid the scheduler from hoisting batch b+1's
            # engine work (v-adds, bias copies, epilogues) into batch b's
            # streams — cross-batch hoists invert priorities on the in-order
            # engines. Work for batch b+1 emitted in section b (stage-A
            # prefetch) intentionally shares tier b.
            ctx_b = tc.tile_wait_until(b)
            ctx_b.__enter__()
            # every section preps the NEXT batch's x: (b+1)%b_pc — section
            # b_pc-1 preps batch 0 of the next rep-loop iteration, so the
            # loop back-edge barrier costs no refill (software pipelining
            # across the For_i back edge; the prologue primes iteration 1)
            for t in range(n_s):
                emit_load((b + 1) % b_pc, t)
            # ---- stage B (pure projections), with the next batch's
            # convert+transpose interleaved (xbf slots are free: batch b's
            # transposes ran during B(b-1); Pool/DMA are idle during B) ----
            for t in range(n_s):
                emit_conv_xbar(t)
                emit_xbar(b + 1, t)
            proj_chunk(b, "q", 0)
            proj_chunk(b, "k", 0)
            proj_chunk(b, "q", 1)
            proj_chunk(b, "k", 1)
            # ---- stage C/D with one tile of score lookahead, interleaved
            # with the NEXT batch's v tiles (their xT landed during B(b)):
            # ~1.7us of independent PE work per pv iteration covers the
            # cross-engine mask->exp->pv latency chains ----
            scores_tile(0)
            for i in range(n_s):
                if i + 1 < n_s:
                    scores_tile(i + 1)
                    v_tile(b + 1, i)
                    pv_tile(b, i)
                else:
                    # last iteration: pv first, then v — the v matmuls cover
                    # pv(7)'s epilogue/out drain so the next section's
                    # projections start against drained PSUM banks
                    pv_tile(b, i)
                    v_tile(b + 1, i)
            ctx_b.__exit__(None, None, None)

    nc.compile()
    return nc


_BUILT = {}


def _get_nc(b_pc, s, e, a):
    key = (b_pc, s, e, a)
    if key not in _BUILT:
        _BUILT[key] = build(b_pc, s, e, a)
    return _BUILT[key]


def run_sharded(inputs, b_pc, s, e, a, **run_kwargs):
    """Run the SPMD kernel over N_CORES cores, sharding batch dim of x."""
    x = np.ascontiguousarray(inputs["x"], dtype=np.float32)
    b_total = x.shape[0]
    assert b_total == b_pc * N_CORES
    shared = {
        "Wq": np.ascontiguousarray(inputs["Wq"], dtype=np.float32),
        "Wk": np.ascontiguousarray(inputs["Wk"], dtype=np.float32),
        "Wv": np.ascontiguousarray(inputs["Wv"], dtype=np.float32),
        "bq": np.ascontiguousarray(inputs["bq"], dtype=np.float32),
        "bk": np.ascontiguousarray(inputs["bk"], dtype=np.float32),
        "bv": np.ascontiguousarray(inputs["bv"], dtype=np.float32),
    }
    in_maps = [
        {"x": x[c * b_pc:(c + 1) * b_pc], **shared} for c in range(N_CORES)
    ]
    nc = _get_nc(b_pc, s, e, a)
    res = run_bass_kernel_spmd(nc, in_maps, core_ids=list(range(N_CORES)),
                               **run_kwargs)
    full = np.concatenate([res.results[c]["out"] for c in range(N_CORES)], axis=0)
    return full, res


def kernel(**inputs) -> np.ndarray:
    out, _ = run_sharded(inputs, B // N_CORES, S, E, A)
    return out


# revision 44
# speedup vs baseline: 1.5012x; 1.0645x over previous
"""Masked self-attention Trainium2 Bass kernel.

Reference computation (per batch b):
    q = x @ Wq + bq ; k = x @ Wk + bk ; v = x @ Wv + bv      # [S, A]
    scores = (q @ k.T) / sqrt(S)  with causal mask            # [S, S]
    out = softmax(scores, axis=-1) @ v                        # [S, A]

Sharding: data-parallel over batch across 8 NeuronCores (B=32 -> 4 per core),
weights replicated. No collectives.

Per-core design. Mixed precision: xT / W / expT / v are bf16; qT / kT are
fp8e4 feeding DoubleRow score matmuls (2 MACs/cell/cycle, contraction 256 of
A per matmul); PSUM accumulation and DRAM-facing input/output stay fp32.
Measured rel err ~1.0e-2 vs the 2e-2 gate (fp8 q/k rounding dominates; the
softmax numerator/denominator share the same rounded weights so most of the
exp error cancels).

The PE never transposes: x [S,E] fp32 is DMA'd to SBUF, converted fp32->bf16
on Pool/DVE, and transposed bf16 SBUF->SBUF by the XBAR DMA-transpose unit
(InstDmaTransposeAnt, out[p,u,c] = in[c, u*128+p]) straight into the xT
layout the projections consume. xT is stored s-tile-major [P, 4, n_e, P] per
s-half so each transpose writes a contiguous [P, n_e*P] slice (XBAR requires
contiguous destinations); projection matmuls read [:, :, u, :] 3D moving APs
spanning the 4 s-tiles. The last s-tile (104 valid rows) rides a bf16 tile
whose tail rows are zeroed once, so xT cols 1000:1024 are clean zeros.

  Stage A: per s-tile: 2-chunk DMA load (sync queue) -> fp32->bf16 copy
           (Pool; DVE/Pool alternating for batch 0 where DVE is idle) ->
           XBAR transpose (sync queue) into xT.
  Stage B: qT/kT = W.T @ xT -> fp8 DoubleRow pair tiles [P, 2, 1024] with
           bias folded into the ACT PSUM->SBUF copy; q/k stay UNSCALED
           (1/sqrt(S) rides the exp's scale). v = xT.T @ Wv -> [S, A+2] bf16
           with bv added on DVE; the last two columns are constant ones
           (written once) that make the PV matmul emit softmax row-sums.
           Order: v(0..3), q-half0, k-half0, v(4..7), q-half1, k-half1 so
           batch 0's PE work starts after a single transposed tile.
  Stage C: scoresT[k,q] = kT.T @ qT per k-tile in causal-trimmed chunks of
           2 DoubleRow matmuls; exp(scale=1/sqrt(S)) on ACT -> bf16 expT;
           the diagonal block's upper triangle is zeroed in expT by a Pool
           affine_select (no PSUM mask add, no masked-scores overflow risk:
           |scores|/sqrt(S) <~ 5). No max-subtraction.
  Stage D: interleaved with C with one tile of score lookahead: out_psum =
           sum_t expT[t].T @ v_aug[t] in two column chunks on double-buffered
           PSUM banks; DVE reciprocal of the ones-column row-sum scales both
           halves; DMA out per 256-column half (sync queue).

Cross-batch software pipelining: x loads for batch b+1 are emitted at the
start of stage B(b) (they fill during B/C/D); the bf16 converts + XBAR
transposes for batch b+1 are interleaved into batch b's C/D emission (one
s-tile per pv iteration) so the in-order Pool/sync streams stay dense and
batch b+1's projections find xT ready. xT double-buffers by batch parity.
"""

import numpy as np
from contextlib import ExitStack

import concourse.bass as bass
import concourse.mybir as mybir
import concourse.tile as tile
from concourse import bacc
from concourse.bass_utils import run_bass_kernel_spmd

P = 128
F32 = mybir.dt.float32
BF16 = mybir.dt.bfloat16
FP8 = mybir.dt.float8e4
DR = mybir.MatmulPerfMode.DoubleRow
AF = mybir.ActivationFunctionType

N_CORES = 8
B, S, E, A = 32, 1000, 1024, 512


def _even_chunks(start, total, maxc):
    """Split [start, start+total) into ceil(total/maxc) near-even chunks,
    each of even size (required by DoubleRow moving dim)."""
    assert total % 2 == 0
    n = max(1, -(-total // maxc))
    bounds = [start + ((i * total) // n) // 2 * 2 for i in range(n)]
    bounds.append(start + total)
    return [(bounds[i], bounds[i + 1] - bounds[i]) for i in range(n)]


def build(b_pc, s, e, a, reps=1, warm_weights=False):
    # warm_weights: skip the DRAM weight loads (timing-only simulation; the
    # measured rep-loop slope never sees the weight-load transient).
    assert e % P == 0 and a % P == 0
    n_s = -(-s // P)
    n_e = e // P
    n_a = a // P
    assert n_s == 8 and n_e == 8 and n_a == 4
    inv_den = float(s) ** -0.5
    s_tiles = [(t * P, min(P, s - t * P)) for t in range(n_s)]
    s_pad = n_s * P  # 1024; cols s..s_pad are zeros (zero-tail bf16 tile)
    h = a // 2  # PV column split: [0,h) and [h, a+2)
    nt_h = n_s // 2  # s-tiles per half

    nc = bacc.Bacc("TRN2")
    x = nc.dram_tensor("x", [b_pc, s, e], F32, kind="ExternalInput").ap()
    w_dram = {
        "q": nc.dram_tensor("Wq", [e, a], F32, kind="ExternalInput").ap(),
        "k": nc.dram_tensor("Wk", [e, a], F32, kind="ExternalInput").ap(),
        "v": nc.dram_tensor("Wv", [e, a], F32, kind="ExternalInput").ap(),
    }
    b_dram = {
        "q": nc.dram_tensor("bq", [a], F32, kind="ExternalInput").ap(),
        "k": nc.dram_tensor("bk", [a], F32, kind="ExternalInput").ap(),
        "v": nc.dram_tensor("bv", [a], F32, kind="ExternalInput").ap(),
    }
    out = nc.dram_tensor("out", [b_pc, s, a], F32, kind="ExternalOutput").ap()

    with tile.TileContext(nc) as tc, ExitStack() as ctx:
        pool = ctx.enter_context(tc.tile_pool(name="sb", bufs=1))
        pp_proj = ctx.enter_context(tc.tile_pool(name="pp_proj", bufs=2, space="PSUM"))
        pp_score = ctx.enter_context(tc.tile_pool(name="pp_sc", bufs=2, space="PSUM"))
        pp_o1 = ctx.enter_context(tc.tile_pool(name="pp_o1", bufs=2, space="PSUM"))
        pp_o2 = ctx.enter_context(tc.tile_pool(name="pp_o2", bufs=2, space="PSUM"))

        # ---------------- weights / biases ----------------
        w_sb = {}
        warm_engs = [nc.vector, nc.gpsimd]
        for wi, nm in enumerate(("q", "k", "v")):
            tiles = []
            for u in range(n_e):
                w_r = pool.tile([P, a], BF16, tag=f"w_{nm}", bufs=n_e)
                if warm_weights:
                    # timing-only simulation: weights counted as resident
                    # (memsets spread across engines so no single engine's
                    # stream delays batch 0; first-exec-only transient anyway)
                    warm_engs[(wi * n_e + u) % 2].memset(w_r, 0.0)
                    tiles.append(w_r)
                    continue
                w_stage = pool.tile([P, a], F32, tag="w_stage", bufs=2)
                nc.gpsimd.dma_start(
                    w_stage[:], w_dram[nm][u * P:(u + 1) * P, :])
                nc.vector.tensor_copy(w_r[:], w_stage[:])
                tiles.append(w_r)
            w_sb[nm] = tiles

        bias_sb = {}
        for nm in ("q", "k"):
            b_st = pool.tile([P, n_a], F32, tag=f"b_{nm}", bufs=1)
            nc.gpsimd.dma_start(
                b_st[:], b_dram[nm].rearrange("(m p) -> p m", p=P)
            )
            bias_sb[nm] = b_st
        bv_stage = pool.tile([1, a], F32)
        nc.gpsimd.dma_start(bv_stage[:], b_dram["v"][:])
        bv_bc = pool.tile([P, a], F32)
        nc.gpsimd.partition_broadcast(bv_bc[:], bv_stage[:])

        # additive causal mask for the diagonal block:
        # keep 0 where col q >= row k, else fill -1e9
        amask = pool.tile([P, P], F32)
        nc.gpsimd.memset(amask, 0.0)
        nc.gpsimd.affine_select(
            out=amask, in_=amask,
            compare_op=mybir.AluOpType.is_ge,
            fill=-1.0e9, base=0,
            pattern=[[1, P]], channel_multiplier=-1,
        )



        # ---------------- persistent per-tile SBUF slots ----------------
        x_slots = [pool.tile([P, e], F32, tag=f"x{t}", bufs=1, name=f"x{t}")
                   for t in range(n_s)]
        xbf = [pool.tile([P, e], BF16, tag=f"xbf{t}", bufs=1, name=f"xbf{t}")
               for t in range(n_s)]
        # zero the last tile's invalid rows once: transposes always read the
        # full 128 rows, so xT cols s..s_pad become clean zeros.
        if s_tiles[-1][1] < P:
            nc.gpsimd.memset(xbf[-1][:], 0.0)
        # xT[par][hi]: s-tile-major [P, nt_h, n_e, P]; par = batch parity
        xT = [
            [pool.tile([P, nt_h, n_e, P], BF16, tag=f"xT{par}_{hi}", bufs=1,
                        name=f"xT{par}_{hi}")
             for hi in range(2)]
            for par in range(2)
        ]
        # v as fp8 DoubleRow pair tiles [P, 2, a+2] (k-tiles 2p2, 2p2+1 on
        # the pair dim) with two constant ones-columns (softmax row-sums via
        # the PV matmul); double-buffered by batch parity: v(b+1) is computed
        # during C/D(b), which still reads v(b). Zeroed once so the unwritten
        # tail rows of the last k-tile contribute exact zeros.
        n_p2 = n_s // 2
        va_pad = -(-(a + 2) // 16) * 16  # DR pair-dim byte step must be %16
        vP = [
            [pool.tile([P, 2, va_pad], FP8, tag=f"vP{par}_{p2}", bufs=1,
                       name=f"vP{par}_{p2}")
             for p2 in range(n_p2)]
            for par in range(2)
        ]
        # pv-tile 0 (output rows 0..127) runs in bf16: its softmax support is
        # tiny (row r averages r+1 values), so fp8 v/exp rounding would land
        # nearly unaveraged in the output
        v0_bf = [pool.tile([P, a + 2], BF16, tag=f"v0bf{par}", bufs=1,
                           name=f"v0bf{par}")
                 for par in range(2)]
        for par in range(2):
            for p2 in range(n_p2):
                nc.gpsimd.memset(vP[par][p2][:], 0.0)
                nc.gpsimd.memset(vP[par][p2][:, :, a:a + 2], 1.0)
            nc.gpsimd.memset(v0_bf[par][:, a:a + 2], 1.0)
        # qT/kT fp8 DoubleRow pair tiles: a-tiles (2m, 2m+1) on the pair dim
        qkT = {
            nm: [pool.tile([P, 2, s_pad], FP8, tag=f"{nm}P{m2}", bufs=1,
                           name=f"{nm}P{m2}")
                 for m2 in range(n_a // 2)]
            for nm in ("q", "k")
        }
        # expT as fp8 DoubleRow pair tiles [P, 2, s - 256*p2]; pair-dim j
        # holds k-tiles (2*p2, 2*p2+1), columns are q - 256*p2. Zeroed once:
        # j=1's first 128 columns (the causally-masked sub-diagonal block of
        # tile 2*p2+1) and its unwritten tail rows stay exact zeros, which
        # lets every pv accumulate whole pairs including the diagonal one.
        expP = [pool.tile([P, 2, -(-(s - 2 * P * p2) // 16) * 16], FP8,
                          tag=f"expP{p2}", bufs=1, name=f"expP{p2}")
                for p2 in range(n_p2)]
        for p2 in range(n_p2):
            nc.gpsimd.memset(expP[p2][:, 1, :], 0.0)
        # bf16 copy of tile 0's diagonal exp block for the bf16 pv(0)
        et0_bf = pool.tile([P, P], BF16, tag="et0bf", bufs=1, name="et0_bf")

        # ---------------- emission helpers ----------------
        def emit_load(b, t):
            s0, sl = s_tiles[t]
            wsp = e // 2
            for qi in range(2):
                nc.sync.dma_start(
                    x_slots[t][:sl, qi * wsp:(qi + 1) * wsp],
                    x[b, s0:s0 + sl, qi * wsp:(qi + 1) * wsp],
                )

        def emit_conv_xbar(t, eng=None):
            sl = s_tiles[t][1]
            (eng or nc.gpsimd).tensor_copy(xbf[t][:sl, :], x_slots[t][:sl, :])
            return t

        def emit_xbar(b, t):
            par = b % 2
            nc.sync.dma_start_transpose(
                xT[par][t // nt_h][:, t % nt_h, :, :], xbf[t][:])

        def proj_chunk(b, nm, hi):
            par = b % 2
            c0 = hi * (nt_h * P)
            cl = min(nt_h * P, s - c0)  # trim the s..s_pad padding columns
            n_full = cl // P  # full s-tiles in this half
            tail = cl - n_full * P
            for m in range(n_a):
                mm = pp_proj.tile([P, 512], F32, tag="proj")
                for u in range(n_e):
                    nc.tensor.matmul(
                        mm[:, :n_full * P],
                        w_sb[nm][u][:, m * P:(m + 1) * P],
                        xT[par][hi][:, :n_full, u, :],
                        start=(u == 0), stop=(u == n_e - 1),
                    )
                if tail:
                    for u in range(n_e):
                        nc.tensor.matmul(
                            mm[:, n_full * P:cl],
                            w_sb[nm][u][:, m * P:(m + 1) * P],
                            xT[par][hi][:, n_full, u, :tail],
                            start=(u == 0), stop=(u == n_e - 1),
                        )
                nc.scalar.activation(
                    qkT[nm][m // 2][:, m % 2, c0:c0 + cl], mm[:, :cl],
                    AF.Identity, bias=bias_sb[nm][:, m:m + 1],
                )

        def v_tile(b, t):
            par = b % 2
            s0, sl = s_tiles[t]
            vm = pp_proj.tile([P, 512], F32, tag="proj")
            for u in range(n_e):
                nc.tensor.matmul(
                    vm[:sl, :a],
                    xT[par][t // nt_h][:, t % nt_h, u, :sl],
                    w_sb["v"][u][:],
                    start=(u == 0), stop=(u == n_e - 1),
                )
            nc.vector.tensor_add(
                vP[par][t // 2][:sl, t % 2, :a], vm[:sl, :a], bv_bc[:sl, :])
            if t == 0:
                nc.vector.tensor_add(
                    v0_bf[par][:sl, :a], vm[:sl, :a], bv_bc[:sl, :])

        def scores_tile(t):
            k0, kl = s_tiles[t]
            ep = expP[t // 2]
            e0 = 2 * P * (t // 2)  # column origin of the pair tile
            # the diagonal block is its own small FIRST chunk: pv(t)'s last
            # accumulation pair needs exactly this chunk of expT, so its
            # mask->exp chain must resolve as early as possible
            chunks = [(k0, kl)]
            if k0 + kl < s:
                chunks += _even_chunks(k0 + kl, s - k0 - kl, 512)
            for pi, (c0, cl) in enumerate(chunks):
                sc = pp_score.tile([P, 512], F32, tag="score")
                for m2 in range(n_a // 2):
                    nc.tensor.matmul(
                        sc[:kl, :cl],
                        qkT["k"][m2][:, :, k0:k0 + kl],
                        qkT["q"][m2][:, :, c0:c0 + cl],
                        start=(m2 == 0), stop=(m2 == n_a // 2 - 1),
                        perf_mode=DR,
                    )
                if pi == 0:
                    # diagonal block: additive causal mask in PSUM (DVE)
                    nc.vector.tensor_add(
                        sc[:kl, :kl], sc[:kl, :kl], amask[:kl, :kl]
                    )
                nc.scalar.activation(
                    ep[:kl, t % 2, c0 - e0:c0 - e0 + cl], sc[:kl, :cl],
                    AF.Exp, scale=inv_den,
                )
                if pi == 0 and t == 0:
                    # bf16 diag block for the bf16 pv(0)
                    nc.scalar.activation(
                        et0_bf[:kl, :kl], sc[:kl, :kl], AF.Exp,
                        scale=inv_den,
                    )

        def pv_tile(b, i):
            q0, il = s_tiles[i]
            op1 = pp_o1.tile([P, h], F32, tag="op1")
            op2 = pp_o2.tile([P, a - h + 2], F32, tag="op2")
            par = b % 2
            if i == 0:
                nc.tensor.matmul(op1[:il, :], et0_bf[:, 0:il],
                                 v0_bf[par][:, 0:h], start=True, stop=True)
                nc.tensor.matmul(op2[:il, :], et0_bf[:, 0:il],
                                 v0_bf[par][:, h:a + 2], start=True, stop=True)
            else:
                # fp8 DoubleRow over k-tile pairs; for even i the last pair's
                # j=1 slice (tile i+1) reads the once-zeroed sub-diagonal
                # columns, contributing exact zeros
                np2 = i // 2 + 1
                for p2 in range(np2):
                    c = q0 - 2 * P * p2
                    lhs = expP[p2][:, :, c:c + il]
                    nc.tensor.matmul(
                        op1[:il, :], lhs, vP[par][p2][:, :, 0:h],
                        start=(p2 == 0), stop=(p2 == np2 - 1),
                        perf_mode=DR,
                    )
                    nc.tensor.matmul(
                        op2[:il, :], lhs, vP[par][p2][:, :, h:a + 2],
                        start=(p2 == 0), stop=(p2 == np2 - 1),
                        perf_mode=DR,
                    )
            rec = pool.tile([P, 1], F32, tag="rec", bufs=4, name="rec")
            nc.vector.reciprocal(rec[:il, :], op2[:il, a - h:a - h + 1])
            # deep ring: out DMAs may lag several tiles behind the epilogue
            # (DMA_ENGINES serializes them behind prefetched stage-A traffic)
            o_sb = pool.tile([P, a], F32, tag="o_sb", bufs=8, name="o_sb")
            # epilogue halves on different engines (ACT | DVE) so the
            # op1/op2 PSUM banks drain fast
            # both epilogue halves on DVE: ACT stays free for the exp chain,
            # which paces pv. outs ride the scalar HWDGE queue: everything
            # there is PE-gated, so a hoisted future-batch transpose on the
            # sync queue can never head-block the o_sb ring drain
            nc.vector.tensor_scalar_mul(
                o_sb[:il, 0:h], op1[:il, :], rec[:il, 0:1])
            nc.scalar.dma_start(out[b, q0:q0 + il, 0:h], o_sb[:il, 0:h])
            nc.vector.tensor_scalar_mul(
                o_sb[:il, h:a], op2[:il, 0:a - h], rec[:il, 0:1])
            nc.scalar.dma_start(out[b, q0:q0 + il, h:a], o_sb[:il, h:a])

        # ---------------- per-batch pipeline ----------------
        # prologue: cold stage A for batch 0 plus its v tiles (primes the
        # rotated loop body, which computes v(b+1) during C/D(b)). All loads
        # first (the sync queue pipelines them at full rate), then
        # convert+transpose per tile, converts alternating DVE/Pool.
        for t in range(n_s):
            emit_load(0, t)
        for t in range(n_s):
            emit_conv_xbar(t, nc.vector if t % 2 == 0 else nc.gpsimd)
            emit_xbar(0, t)
            v_tile(0, t)

        rep_ctx = (tc.For_i(0, reps, 1, hint_engines=tuple(nc.engines),
                            staggered_reset=True)
                   if reps > 1 else None)
        if rep_ctx is not None:
            ctx.enter_context(rep_ctx)

        for b in range(b_pc):
            # scheduling tier: forbid the scheduler from hoisting batch b+1's
            # engine work (v-adds, bias copies, epilogues) into batch b's
            # streams — cross-batch hoists invert priorities on the in-order
            # engines. Work for batch b+1 emitted in section b (stage-A
            # prefetch) intentionally shares tier b.
            ctx_b = tc.tile_wait_until(b)
            ctx_b.__enter__()
            # every section preps the NEXT batch's x: (b+1)%b_pc — section
            # b_pc-1 preps batch 0 of the next rep-loop iteration, so the
            # loop back-edge barrier costs no refill (software pipelining
            # across the For_i back edge; the prologue primes iteration 1)
            for t in range(n_s):
                emit_load((b + 1) % b_pc, t)
            # ---- stage B (pure projections), with the next batch's
            # convert+transpose interleaved (xbf slots are free: batch b's
            # transposes ran during B(b-1); Pool/DMA are idle during B) ----
            for t in range(n_s):
                emit_conv_xbar(t)
                emit_xbar(b + 1, t)
            proj_chunk(b, "q", 0)
            proj_chunk(b, "k", 0)
            proj_chunk(b, "q", 1)
            proj_chunk(b, "k", 1)
            # ---- stage C/D with one tile of score lookahead, interleaved
            # with the NEXT batch's v tiles (their xT landed during B(b)):
            # ~1.7us of independent PE work per pv iteration covers the
            # cross-engine mask->exp->pv latency chains ----
            scores_tile(0)
            for i in range(n_s):
                if i + 1 < n_s:
                    scores_tile(i + 1)
                    v_tile(b + 1, i)
                    pv_tile(b, i)
                else:
                    # last iteration: pv first, then v — the v matmuls cover
                    # pv(7)'s epilogue/out drain so the next section's
                    # projections start against drained PSUM banks
                    pv_tile(b, i)
                    v_tile(b + 1, i)
            ctx_b.__exit__(None, None, None)

    nc.compile()
    return nc


_BUILT = {}


def _get_nc(b_pc, s, e, a):
    key = (b_pc, s, e, a)
    if key not in _BUILT:
        _BUILT[key] = build(b_pc, s, e, a)
    return _BUILT[key]


def run_sharded(inputs, b_pc, s, e, a, **run_kwargs):
    """Run the SPMD kernel over N_CORES cores, sharding batch dim of x."""
    x = np.ascontiguousarray(inputs["x"], dtype=np.float32)
    b_total = x.shape[0]
    assert b_total == b_pc * N_CORES
    shared = {
        "Wq": np.ascontiguousarray(inputs["Wq"], dtype=np.float32),
        "Wk": np.ascontiguousarray(inputs["Wk"], dtype=np.float32),
        "Wv": np.ascontiguousarray(inputs["Wv"], dtype=np.float32),
        "bq": np.ascontiguousarray(inputs["bq"], dtype=np.float32),
        "bk": np.ascontiguousarray(inputs["bk"], dtype=np.float32),
        "bv": np.ascontiguousarray(inputs["bv"], dtype=np.float32),
    }
    in_maps = [
        {"x": x[c * b_pc:(c + 1) * b_pc], **shared} for c in range(N_CORES)
    ]
    nc = _get_nc(b_pc, s, e, a)
    res = run_bass_kernel_spmd(nc, in_maps, core_ids=list(range(N_CORES)),
                               **run_kwargs)
    full = np.concatenate([res.results[c]["out"] for c in range(N_CORES)], axis=0)
    return full, res


def kernel(**inputs) -> np.ndarray:
    out, _ = run_sharded(inputs, B // N_CORES, S, E, A)
    return out


# revision 62
# speedup vs baseline: 1.8370x; 1.2237x over previous
"""Masked self-attention Trainium2 Bass kernel.

Reference computation (per batch b):
    q = x @ Wq + bq ; k = x @ Wk + bk ; v = x @ Wv + bv      # [S, A]
    scores = (q @ k.T) / sqrt(S)  with causal mask            # [S, S]
    out = softmax(scores, axis=-1) @ v                        # [S, A]

Sharding: data-parallel over batch across 8 NeuronCores (B=32 -> 4 per core),
weights replicated. No collectives.

Per-core design. Mixed precision: xT / W / expT / v are bf16; qT / kT are
fp8e4 feeding DoubleRow score matmuls (2 MACs/cell/cycle, contraction 256 of
A per matmul); PSUM accumulation and DRAM-facing input/output stay fp32.
Measured rel err ~1.0e-2 vs the 2e-2 gate (fp8 q/k rounding dominates; the
softmax numerator/denominator share the same rounded weights so most of the
exp error cancels).

The PE never transposes: x [S,E] fp32 is DMA'd to SBUF, converted fp32->bf16
on Pool/DVE, and transposed bf16 SBUF->SBUF by the XBAR DMA-transpose unit
(InstDmaTransposeAnt, out[p,u,c] = in[c, u*128+p]) straight into the xT
layout the projections consume. xT is stored s-tile-major [P, 4, n_e, P] per
s-half so each transpose writes a contiguous [P, n_e*P] slice (XBAR requires
contiguous destinations); projection matmuls read [:, :, u, :] 3D moving APs
spanning the 4 s-tiles. The last s-tile (104 valid rows) rides a bf16 tile
whose tail rows are zeroed once, so xT cols 1000:1024 are clean zeros.

  Stage A: per s-tile: 2-chunk DMA load (sync queue) -> fp32->bf16 copy
           (Pool; DVE/Pool alternating for batch 0 where DVE is idle) ->
           XBAR transpose (sync queue) into xT.
  Stage B: qT/kT = W.T @ xT -> fp8 DoubleRow pair tiles [P, 2, 1024] with
           bias folded into the ACT PSUM->SBUF copy; q/k stay UNSCALED
           (1/sqrt(S) rides the exp's scale). v = xT.T @ Wv -> [S, A+2] bf16
           with bv added on DVE; the last two columns are constant ones
           (written once) that make the PV matmul emit softmax row-sums.
           Order: v(0..3), q-half0, k-half0, v(4..7), q-half1, k-half1 so
           batch 0's PE work starts after a single transposed tile.
  Stage C: scoresT[k,q] = kT.T @ qT per k-tile in causal-trimmed chunks of
           2 DoubleRow matmuls; exp(scale=1/sqrt(S)) on ACT -> bf16 expT;
           the diagonal block's upper triangle is zeroed in expT by a Pool
           affine_select (no PSUM mask add, no masked-scores overflow risk:
           |scores|/sqrt(S) <~ 5). No max-subtraction.
  Stage D: interleaved with C with one tile of score lookahead: out_psum =
           sum_t expT[t].T @ v_aug[t] in two column chunks on double-buffered
           PSUM banks; DVE reciprocal of the ones-column row-sum scales both
           halves; DMA out per 256-column half (sync queue).

Cross-batch software pipelining: x loads for batch b+1 are emitted at the
start of stage B(b) (they fill during B/C/D); the bf16 converts + XBAR
transposes for batch b+1 are interleaved into batch b's C/D emission (one
s-tile per pv iteration) so the in-order Pool/sync streams stay dense and
batch b+1's projections find xT ready. xT double-buffers by batch parity.
"""

import numpy as np
from contextlib import ExitStack

import concourse.bass as bass
import concourse.mybir as mybir
import concourse.tile as tile
from concourse import bacc
from concourse.bass_utils import run_bass_kernel_spmd

P = 128
F32 = mybir.dt.float32
BF16 = mybir.dt.bfloat16
FP8 = mybir.dt.float8e4
DR = mybir.MatmulPerfMode.DoubleRow
AF = mybir.ActivationFunctionType

N_CORES = 8
B, S, E, A = 32, 1000, 1024, 512


def _even_chunks(start, total, maxc):
    """Split [start, start+total) into ceil(total/maxc) near-even chunks,
    each of even size (required by DoubleRow moving dim)."""
    assert total % 2 == 0
    n = max(1, -(-total // maxc))
    bounds = [start + ((i * total) // n) // 2 * 2 for i in range(n)]
    bounds.append(start + total)
    return [(bounds[i], bounds[i + 1] - bounds[i]) for i in range(n)]


def build(b_pc, s, e, a, reps=1, warm_weights=False):
    # warm_weights: skip the DRAM weight loads (timing-only simulation; the
    # measured rep-loop slope never sees the weight-load transient).
    assert e % P == 0 and a % P == 0
    n_s = -(-s // P)
    n_e = e // P
    n_a = a // P
    assert n_s == 8 and n_e == 8 and n_a == 4
    inv_den = float(s) ** -0.5
    s_tiles = [(t * P, min(P, s - t * P)) for t in range(n_s)]
    s_pad = n_s * P  # 1024; cols s..s_pad are zeros (zero-tail bf16 tile)
    h = a // 2  # PV column split: [0,h) and [h, a+2)
    nt_h = n_s // 2  # s-tiles per half

    nc = bacc.Bacc("TRN2")
    x = nc.dram_tensor("x", [b_pc, s, e], F32, kind="ExternalInput").ap()
    w_dram = {
        "q": nc.dram_tensor("Wq", [e, a], F32, kind="ExternalInput").ap(),
        "k": nc.dram_tensor("Wk", [e, a], F32, kind="ExternalInput").ap(),
        "v": nc.dram_tensor("Wv", [e, a], F32, kind="ExternalInput").ap(),
    }
    b_dram = {
        "q": nc.dram_tensor("bq", [a], F32, kind="ExternalInput").ap(),
        "k": nc.dram_tensor("bk", [a], F32, kind="ExternalInput").ap(),
        "v": nc.dram_tensor("bv", [a], F32, kind="ExternalInput").ap(),
    }
    out = nc.dram_tensor("out", [b_pc, s, a], F32, kind="ExternalOutput").ap()

    with tile.TileContext(nc) as tc, ExitStack() as ctx:
        pool = ctx.enter_context(tc.tile_pool(name="sb", bufs=1))
        pp_proj = ctx.enter_context(tc.tile_pool(name="pp_proj", bufs=2, space="PSUM"))
        pp_score = ctx.enter_context(tc.tile_pool(name="pp_sc", bufs=2, space="PSUM"))
        pp_o1 = ctx.enter_context(tc.tile_pool(name="pp_o1", bufs=2, space="PSUM"))
        pp_o2 = ctx.enter_context(tc.tile_pool(name="pp_o2", bufs=2, space="PSUM"))

        # ---------------- weights / biases ----------------
        # w_sb (bf16): feeds the bf16 paths (q/k diagonal-block projections
        # and the v projection of s-tile 0). w8 (fp8, scaled by 32 to clear
        # the e4m3 subnormal range): DoubleRow pair tiles [P, 2, a] with
        # e-tiles (2*u2, 2*u2+1) on the pair dim, feeding the fp8 q/k/v
        # projections; the 1/32 rides the PSUM->SBUF activation scale.
        w_sb = {}
        w8 = {}
        warm_engs = [nc.vector, nc.gpsimd]
        for wi, nm in enumerate(("q", "k", "v")):
            tiles = []
            tiles8 = [pool.tile([P, 2, a], FP8, tag=f"w8_{nm}{u2}", bufs=1,
                                name=f"w8_{nm}{u2}")
                      for u2 in range(n_e // 2)]
            for u in range(n_e):
                w_r = pool.tile([P, a], BF16, tag=f"w_{nm}", bufs=n_e)
                if warm_weights:
                    # timing-only simulation: weights counted as resident
                    # (memsets spread across engines so no single engine's
                    # stream delays batch 0; first-exec-only transient anyway)
                    warm_engs[(wi * n_e + u) % 2].memset(w_r, 0.0)
                    if u % 2 == 0:
                        warm_engs[u % 2].memset(tiles8[u // 2], 0.0)
                    tiles.append(w_r)
                    continue
                w_stage = pool.tile([P, a], F32, tag="w_stage", bufs=2)
                nc.gpsimd.dma_start(
                    w_stage[:], w_dram[nm][u * P:(u + 1) * P, :])
                nc.vector.tensor_copy(w_r[:], w_stage[:])
                if nm == "v":
                    # unscaled: v's fp8 subnormal W error averages out in PV
                    nc.vector.tensor_copy(
                        tiles8[u // 2][:, u % 2, :], w_stage[:])
                else:
                    nc.vector.tensor_scalar_mul(
                        tiles8[u // 2][:, u % 2, :], w_stage[:], 32.0)
                tiles.append(w_r)
            w_sb[nm] = tiles
            w8[nm] = tiles8

        bias_sb = {}
        for nm in ("q", "k"):
            b_st = pool.tile([P, n_a], F32, tag=f"b_{nm}", bufs=1)
            nc.gpsimd.dma_start(
                b_st[:], b_dram[nm].rearrange("(m p) -> p m", p=P)
            )
            bias_sb[nm] = b_st
        bv_stage = pool.tile([1, a], F32)
        nc.gpsimd.dma_start(bv_stage[:], b_dram["v"][:])
        bv_bc = pool.tile([P, a], F32)
        nc.gpsimd.partition_broadcast(bv_bc[:], bv_stage[:])


        # additive causal mask for the diagonal block:
        # keep 0 where col q >= row k, else fill -1e9
        amask = pool.tile([P, P], F32)
        nc.gpsimd.memset(amask, 0.0)
        nc.gpsimd.affine_select(
            out=amask, in_=amask,
            compare_op=mybir.AluOpType.is_ge,
            fill=-1.0e9, base=0,
            pattern=[[1, P]], channel_multiplier=-1,
        )



        # ---------------- persistent per-tile SBUF slots ----------------
        x_slots = [pool.tile([P, e], F32, tag=f"x{t}", bufs=1, name=f"x{t}")
                   for t in range(n_s)]
        xbf = [pool.tile([P, e], BF16, tag=f"xbf{t}", bufs=1, name=f"xbf{t}")
               for t in range(n_s)]
        # zero the last tile's invalid rows once: transposes always read the
        # full 128 rows, so xT cols s..s_pad become clean zeros.
        if s_tiles[-1][1] < P:
            nc.gpsimd.memset(xbf[-1][:], 0.0)
        # xT[par][hi]: s-tile-major [P, nt_h, n_e, P]; par = batch parity.
        # xT8: fp8 copy feeding the DoubleRow projections (e-tile pairs
        # (2*u2, 2*u2+1) are adjacent on the u axis, so [:, t, 2u2:2u2+2, :]
        # is already a valid DR pair slice with a 256B pair stride).
        xT = [
            [pool.tile([P, nt_h, n_e, P], BF16, tag=f"xT{par}_{hi}", bufs=1,
                        name=f"xT{par}_{hi}")
             for hi in range(2)]
            for par in range(2)
        ]
        # xT8 layout is u-major [P, u2, 2, nt_h*P]: per (u2) the DR moving
        # slice [:, u2, :, :] spans the half's s-columns contiguously, so a
        # projection m-chunk is ONE accumulation chain over u2 on a full
        # PSUM bank (HW zeroes the whole bank at chain start, so per-region
        # sub-chains in one bank are illegal)
        xT8 = [
            [pool.tile([P, n_e // 2, 2, nt_h * P], FP8,
                       tag=f"xT8{par}_{hi}", bufs=1, name=f"xT8{par}_{hi}")
             for hi in range(2)]
            for par in range(2)
        ]
        # v as fp8 DoubleRow pair tiles [P, 2, a+2] (k-tiles 2p2, 2p2+1 on
        # the pair dim) with two constant ones-columns (softmax row-sums via
        # the PV matmul); double-buffered by batch parity: v(b+1) is computed
        # during C/D(b), which still reads v(b). Zeroed once so the unwritten
        # tail rows of the last k-tile contribute exact zeros.
        n_p2 = n_s // 2
        va_pad = -(-(a + 2) // 16) * 16  # DR pair-dim byte step must be %16
        vP = [
            [pool.tile([P, 2, va_pad], FP8, tag=f"vP{par}_{p2}", bufs=1,
                       name=f"vP{par}_{p2}")
             for p2 in range(n_p2)]
            for par in range(2)
        ]
        # pv-tile 0 (output rows 0..127) runs in bf16: its softmax support is
        # tiny (row r averages r+1 values), so fp8 v/exp rounding would land
        # nearly unaveraged in the output
        v0_bf = [pool.tile([P, a + 2], BF16, tag=f"v0bf{par}", bufs=1,
                           name=f"v0bf{par}")
                 for par in range(2)]
        for par in range(2):
            for p2 in range(n_p2):
                nc.gpsimd.memset(vP[par][p2][:], 0.0)
                nc.gpsimd.memset(vP[par][p2][:, :, a:a + 2], 1.0)
            nc.gpsimd.memset(v0_bf[par][:, a:a + 2], 1.0)
        # qT/kT fp8 DoubleRow pair tiles: a-tiles (2m, 2m+1) on the pair dim
        qkT = {
            nm: [pool.tile([P, 2, s_pad], FP8, tag=f"{nm}P{m2}", bufs=1,
                           name=f"{nm}P{m2}")
                 for m2 in range(n_a // 2)]
            for nm in ("q", "k")
        }
        # expT as fp8 DoubleRow pair tiles [P, 2, s - 256*p2]; pair-dim j
        # holds k-tiles (2*p2, 2*p2+1), columns are q - 256*p2. Zeroed once:
        # j=1's first 128 columns (the causally-masked sub-diagonal block of
        # tile 2*p2+1) and its unwritten tail rows stay exact zeros, which
        # lets every pv accumulate whole pairs including the diagonal one.
        expP = [pool.tile([P, 2, -(-(s - 2 * P * p2) // 16) * 16], FP8,
                          tag=f"expP{p2}", bufs=1, name=f"expP{p2}")
                for p2 in range(n_p2)]
        for p2 in range(n_p2):
            nc.gpsimd.memset(expP[p2][:, 1, :], 0.0)
        # bf16 copy of tile 0's diagonal exp block for the bf16 pv(0)
        et0_bf = pool.tile([P, P], BF16, tag="et0bf", bufs=1, name="et0_bf")
        # bf16-projected q/k for s-tile 0 ([P(a-part), m, P]): output rows
        # q < 128 average few softmax terms, so their scores bypass fp8
        # entirely (this also removes the old fp8 q/k rounding error there)
        qT0_bf = pool.tile([P, n_a, P], BF16, tag="qT0bf", bufs=1,
                           name="qT0_bf")
        kT0_bf = pool.tile([P, n_a, P], BF16, tag="kT0bf", bufs=1,
                           name="kT0_bf")

        # ---------------- emission helpers ----------------
        def emit_load(b, t):
            s0, sl = s_tiles[t]
            wsp = e // 2
            for qi in range(2):
                nc.sync.dma_start(
                    x_slots[t][:sl, qi * wsp:(qi + 1) * wsp],
                    x[b, s0:s0 + sl, qi * wsp:(qi + 1) * wsp],
                )

        def emit_conv_xbar(t, eng=None):
            sl = s_tiles[t][1]
            (eng or nc.gpsimd).tensor_copy(xbf[t][:sl, :], x_slots[t][:sl, :])
            return t

        def emit_xbar(b, t):
            par = b % 2
            hi = t // nt_h
            nc.sync.dma_start_transpose(
                xT[par][hi][:, t % nt_h, :, :], xbf[t][:])
            if t % nt_h == nt_h - 1:
                # half complete: emit the fp8 u-major repack for the
                # DoubleRow projections (one 4D copy per e-tile pair)
                for u2 in range(n_e // 2):
                    nc.vector.tensor_copy(
                        xT8[par][hi][:, u2, :, :].rearrange(
                            "p j (t c) -> p j t c", c=P),
                        xT[par][hi][:, :, 2 * u2:2 * u2 + 2, :].rearrange(
                            "p t j c -> p j t c"),
                    )

        def proj_chunk(b, nm, hi):
            # fp8 DoubleRow projection: 4 matmuls of 256-contraction per
            # (m, s-tile); per u2 the stationary W pair is loaded once and
            # streams the half's 4 s-tiles
            par = b % 2
            c0 = hi * (nt_h * P)
            cl_h = min(nt_h * P, s - c0)  # trim the s..s_pad padding columns
            for m in range(n_a):
                mm = pp_proj.tile([P, 512], F32, tag="proj")
                for u2 in range(n_e // 2):
                    nc.tensor.matmul(
                        mm[:, :cl_h],
                        w8[nm][u2][:, :, m * P:(m + 1) * P],
                        xT8[par][hi][:, u2, :, :cl_h],
                        start=(u2 == 0), stop=(u2 == n_e // 2 - 1),
                        perf_mode=DR,
                    )
                nc.scalar.activation(
                    qkT[nm][m // 2][:, m % 2, c0:c0 + cl_h], mm[:, :cl_h],
                    AF.Identity, bias=bias_sb[nm][:, m:m + 1],
                    scale=1.0 / 32.0,
                )

        def proj_diag(b):
            # bf16 q/k projections of s-tile 0 only, for the diagonal block
            par = b % 2
            for nm, dst in (("q", qT0_bf), ("k", kT0_bf)):
                for m in range(n_a):
                    mm = pp_score.tile([P, 512], F32, tag="score")
                    for u in range(n_e):
                        nc.tensor.matmul(
                            mm[:, :P],
                            w_sb[nm][u][:, m * P:(m + 1) * P],
                            xT[par][0][:, 0, u, :],
                            start=(u == 0), stop=(u == n_e - 1),
                        )
                    nc.scalar.activation(
                        dst[:, m, :], mm[:, :P],
                        AF.Identity, bias=bias_sb[nm][:, m:m + 1],
                    )

        def v_tile(b, t):
            par = b % 2
            s0, sl = s_tiles[t]
            vm = pp_proj.tile([P, 512], F32, tag="proj")
            if t == 0:
                # bf16 projection: rows < 128 read v (almost) unaveraged
                for u in range(n_e):
                    nc.tensor.matmul(
                        vm[:sl, :a],
                        xT[par][0][:, 0, u, :sl],
                        w_sb["v"][u][:],
                        start=(u == 0), stop=(u == n_e - 1),
                    )
                nc.vector.tensor_add(
                    vP[par][0][:sl, 0, :a], vm[:sl, :a], bv_bc[:sl, :])
                nc.vector.tensor_add(
                    v0_bf[par][:sl, :a], vm[:sl, :a], bv_bc[:sl, :])
                return
            # fp8 DoubleRow projection (unscaled Wv: the subnormal-range W
            # quantization error averages out across the softmax support)
            tl = t % nt_h
            for u2 in range(n_e // 2):
                nc.tensor.matmul(
                    vm[:sl, :a],
                    xT8[par][t // nt_h][:, u2, :, tl * P:tl * P + sl],
                    w8["v"][u2][:],
                    start=(u2 == 0), stop=(u2 == n_e // 2 - 1),
                    perf_mode=DR,
                )
            nc.vector.tensor_add(
                vP[par][t // 2][:sl, t % 2, :a], vm[:sl, :a], bv_bc[:sl, :])

        def scores_tile(t):
            k0, kl = s_tiles[t]
            ep = expP[t // 2]
            e0 = 2 * P * (t // 2)  # column origin of the pair tile
            # the diagonal block is its own small FIRST chunk: pv(t)'s last
            # accumulation pair needs exactly this chunk of expT, so its
            # mask->exp chain must resolve as early as possible
            chunks = [(k0, kl)]
            if k0 + kl < s:
                chunks += _even_chunks(k0 + kl, s - k0 - kl, 512)
            for pi, (c0, cl) in enumerate(chunks):
                sc = pp_score.tile([P, 512], F32, tag="score")
                if pi == 0 and t == 0:
                    # tile 0's diagonal block in bf16 (rows q < 128): feeds
                    # the bf16 pv(0) only; the fp8 pair tile's copy of this
                    # region is never read
                    for m in range(n_a):
                        nc.tensor.matmul(
                            sc[:kl, :kl],
                            kT0_bf[:, m, :], qT0_bf[:, m, :],
                            start=(m == 0), stop=(m == n_a - 1),
                        )
                    nc.vector.tensor_add(
                        sc[:kl, :kl], sc[:kl, :kl], amask[:kl, :kl]
                    )
                    nc.scalar.activation(
                        et0_bf[:kl, :kl], sc[:kl, :kl], AF.Exp,
                        scale=inv_den,
                    )
                    continue
                for m2 in range(n_a // 2):
                    nc.tensor.matmul(
                        sc[:kl, :cl],
                        qkT["k"][m2][:, :, k0:k0 + kl],
                        qkT["q"][m2][:, :, c0:c0 + cl],
                        start=(m2 == 0), stop=(m2 == n_a // 2 - 1),
                        perf_mode=DR,
                    )
                if pi == 0:
                    # diagonal block: additive causal mask in PSUM (DVE)
                    nc.vector.tensor_add(
                        sc[:kl, :kl], sc[:kl, :kl], amask[:kl, :kl]
                    )
                nc.scalar.activation(
                    ep[:kl, t % 2, c0 - e0:c0 - e0 + cl], sc[:kl, :cl],
                    AF.Exp, scale=inv_den,
                )

        def pv_tile(b, i):
            q0, il = s_tiles[i]
            op1 = pp_o1.tile([P, h], F32, tag="op1")
            op2 = pp_o2.tile([P, a - h + 2], F32, tag="op2")
            par = b % 2
            if i == 0:
                nc.tensor.matmul(op1[:il, :], et0_bf[:, 0:il],
                                 v0_bf[par][:, 0:h], start=True, stop=True)
                nc.tensor.matmul(op2[:il, :], et0_bf[:, 0:il],
                                 v0_bf[par][:, h:a + 2], start=True, stop=True)
            else:
                # fp8 DoubleRow over k-tile pairs; for even i the last pair's
                # j=1 slice (tile i+1) reads the once-zeroed sub-diagonal
                # columns, contributing exact zeros
                np2 = i // 2 + 1
                for p2 in range(np2):
                    c = q0 - 2 * P * p2
                    lhs = expP[p2][:, :, c:c + il]
                    nc.tensor.matmul(
                        op1[:il, :], lhs, vP[par][p2][:, :, 0:h],
                        start=(p2 == 0), stop=(p2 == np2 - 1),
                        perf_mode=DR,
                    )
                    nc.tensor.matmul(
                        op2[:il, :], lhs, vP[par][p2][:, :, h:a + 2],
                        start=(p2 == 0), stop=(p2 == np2 - 1),
                        perf_mode=DR,
                    )
            rec = pool.tile([P, 1], F32, tag="rec", bufs=4, name="rec")
            nc.vector.reciprocal(rec[:il, :], op2[:il, a - h:a - h + 1])
            # deep ring: out DMAs may lag several tiles behind the epilogue
            # (DMA_ENGINES serializes them behind prefetched stage-A traffic)
            o_sb = pool.tile([P, a], F32, tag="o_sb", bufs=8, name="o_sb")
            # epilogue halves on different engines (ACT | DVE) so the
            # op1/op2 PSUM banks drain fast
            # both epilogue halves on DVE: ACT stays free for the exp chain,
            # which paces pv. outs ride the scalar HWDGE queue: everything
            # there is PE-gated, so a hoisted future-batch transpose on the
            # sync queue can never head-block the o_sb ring drain
            nc.vector.tensor_scalar_mul(
                o_sb[:il, 0:h], op1[:il, :], rec[:il, 0:1])
            nc.scalar.dma_start(out[b, q0:q0 + il, 0:h], o_sb[:il, 0:h])
            nc.vector.tensor_scalar_mul(
                o_sb[:il, h:a], op2[:il, 0:a - h], rec[:il, 0:1])
            nc.scalar.dma_start(out[b, q0:q0 + il, h:a], o_sb[:il, h:a])

        # ---------------- per-batch pipeline ----------------
        # prologue: cold stage A for batch 0 plus its v tiles (primes the
        # rotated loop body, which computes v(b+1) during C/D(b)). All loads
        # first (the sync queue pipelines them at full rate), then
        # convert+transpose per tile, converts alternating DVE/Pool.
        for t in range(n_s):
            emit_load(0, t)
        for t in range(n_s):
            emit_conv_xbar(t, nc.vector if t % 2 == 0 else nc.gpsimd)
            emit_xbar(0, t)
        for t in range(n_s):
            v_tile(0, t)

        rep_ctx = (tc.For_i(0, reps, 1, hint_engines=tuple(nc.engines),
                            staggered_reset=True)
                   if reps > 1 else None)
        if rep_ctx is not None:
            ctx.enter_context(rep_ctx)

        for b in range(b_pc):
            # scheduling tier: forbid the scheduler from hoisting batch b+1's
            # engine work (v-adds, bias copies, epilogues) into batch b's
            # streams — cross-batch hoists invert priorities on the in-order
            # engines. Work for batch b+1 emitted in section b (stage-A
            # prefetch) intentionally shares tier b.
            ctx_b = tc.tile_wait_until(b)
            ctx_b.__enter__()
            # every section preps the NEXT batch's x: (b+1)%b_pc — section
            # b_pc-1 preps batch 0 of the next rep-loop iteration, so the
            # loop back-edge barrier costs no refill (software pipelining
            # across the For_i back edge; the prologue primes iteration 1)
            for t in range(n_s):
                emit_load((b + 1) % b_pc, t)
            # ---- stage B (pure projections), with the next batch's
            # convert+transpose interleaved (xbf slots are free: batch b's
            # transposes ran during B(b-1); Pool/DMA are idle during B) ----
            for t in range(n_s):
                emit_conv_xbar(t)
                emit_xbar(b + 1, t)
            proj_diag(b)
            proj_chunk(b, "q", 0)
            proj_chunk(b, "k", 0)
            proj_chunk(b, "q", 1)
            proj_chunk(b, "k", 1)
            # ---- stage C/D with one tile of score lookahead, interleaved
            # with the NEXT batch's v tiles (their xT landed during B(b)):
            # ~1.7us of independent PE work per pv iteration covers the
            # cross-engine mask->exp->pv latency chains ----
            scores_tile(0)
            for i in range(n_s):
                if i + 1 < n_s:
                    scores_tile(i + 1)
                    v_tile(b + 1, i)
                    pv_tile(b, i)
                else:
                    # last iteration: pv first, then v — the v matmuls cover
                    # pv(7)'s epilogue/out drain so the next section's
                    # projections start against drained PSUM banks
                    pv_tile(b, i)
                    v_tile(b + 1, i)
            ctx_b.__exit__(None, None, None)

    nc.compile()
    return nc


_BUILT = {}


def _get_nc(b_pc, s, e, a):
    key = (b_pc, s, e, a)
    if key not in _BUILT:
        _BUILT[key] = build(b_pc, s, e, a)
    return _BUILT[key]


def run_sharded(inputs, b_pc, s, e, a, **run_kwargs):
    """Run the SPMD kernel over N_CORES cores, sharding batch dim of x."""
    x = np.ascontiguousarray(inputs["x"], dtype=np.float32)
    b_total = x.shape[0]
    assert b_total == b_pc * N_CORES
    shared = {
        "Wq": np.ascontiguousarray(inputs["Wq"], dtype=np.float32),
        "Wk": np.ascontiguousarray(inputs["Wk"], dtype=np.float32),
        "Wv": np.ascontiguousarray(inputs["Wv"], dtype=np.float32),
        "bq": np.ascontiguousarray(inputs["bq"], dtype=np.float32),
        "bk": np.ascontiguousarray(inputs["bk"], dtype=np.float32),
        "bv": np.ascontiguousarray(inputs["bv"], dtype=np.float32),
    }
    in_maps = [
        {"x": x[c * b_pc:(c + 1) * b_pc], **shared} for c in range(N_CORES)
    ]
    nc = _get_nc(b_pc, s, e, a)
    res = run_bass_kernel_spmd(nc, in_maps, core_ids=list(range(N_CORES)),
                               **run_kwargs)
    full = np.concatenate([res.results[c]["out"] for c in range(N_CORES)], axis=0)
    return full, res


def kernel(**inputs) -> np.ndarray:
    out, _ = run_sharded(inputs, B // N_CORES, S, E, A)
    return out


# revision 66
# speedup vs baseline: 1.8963x; 1.0323x over previous
"""Masked self-attention Trainium2 Bass kernel.

Reference computation (per batch b):
    q = x @ Wq + bq ; k = x @ Wk + bk ; v = x @ Wv + bv      # [S, A]
    scores = (q @ k.T) / sqrt(S)  with causal mask            # [S, S]
    out = softmax(scores, axis=-1) @ v                        # [S, A]

Sharding: data-parallel over batch across 8 NeuronCores (B=32 -> 4 per core),
weights replicated. No collectives.

Per-core design. Mixed precision: xT / W / expT / v are bf16; qT / kT are
fp8e4 feeding DoubleRow score matmuls (2 MACs/cell/cycle, contraction 256 of
A per matmul); PSUM accumulation and DRAM-facing input/output stay fp32.
Measured rel err ~1.0e-2 vs the 2e-2 gate (fp8 q/k rounding dominates; the
softmax numerator/denominator share the same rounded weights so most of the
exp error cancels).

The PE never transposes: x [S,E] fp32 is DMA'd to SBUF, converted fp32->bf16
on Pool/DVE, and transposed bf16 SBUF->SBUF by the XBAR DMA-transpose unit
(InstDmaTransposeAnt, out[p,u,c] = in[c, u*128+p]) straight into the xT
layout the projections consume. xT is stored s-tile-major [P, 4, n_e, P] per
s-half so each transpose writes a contiguous [P, n_e*P] slice (XBAR requires
contiguous destinations); projection matmuls read [:, :, u, :] 3D moving APs
spanning the 4 s-tiles. The last s-tile (104 valid rows) rides a bf16 tile
whose tail rows are zeroed once, so xT cols 1000:1024 are clean zeros.

  Stage A: per s-tile: 2-chunk DMA load (sync queue) -> fp32->bf16 copy
           (Pool; DVE/Pool alternating for batch 0 where DVE is idle) ->
           XBAR transpose (sync queue) into xT.
  Stage B: qT/kT = W.T @ xT -> fp8 DoubleRow pair tiles [P, 2, 1024] with
           bias folded into the ACT PSUM->SBUF copy; q/k stay UNSCALED
           (1/sqrt(S) rides the exp's scale). v = xT.T @ Wv -> [S, A+2] bf16
           with bv added on DVE; the last two columns are constant ones
           (written once) that make the PV matmul emit softmax row-sums.
           Order: v(0..3), q-half0, k-half0, v(4..7), q-half1, k-half1 so
           batch 0's PE work starts after a single transposed tile.
  Stage C: scoresT[k,q] = kT.T @ qT per k-tile in causal-trimmed chunks of
           2 DoubleRow matmuls; exp(scale=1/sqrt(S)) on ACT -> bf16 expT;
           the diagonal block's upper triangle is zeroed in expT by a Pool
           affine_select (no PSUM mask add, no masked-scores overflow risk:
           |scores|/sqrt(S) <~ 5). No max-subtraction.
  Stage D: interleaved with C with one tile of score lookahead: out_psum =
           sum_t expT[t].T @ v_aug[t] in two column chunks on double-buffered
           PSUM banks; DVE reciprocal of the ones-column row-sum scales both
           halves; DMA out per 256-column half (sync queue).

Cross-batch software pipelining: x loads for batch b+1 are emitted at the
start of stage B(b) (they fill during B/C/D); the bf16 converts + XBAR
transposes for batch b+1 are interleaved into batch b's C/D emission (one
s-tile per pv iteration) so the in-order Pool/sync streams stay dense and
batch b+1's projections find xT ready. xT double-buffers by batch parity.
"""

import numpy as np
from contextlib import ExitStack

import concourse.bass as bass
import concourse.mybir as mybir
import concourse.tile as tile
from concourse import bacc
from concourse.bass_utils import run_bass_kernel_spmd

P = 128
F32 = mybir.dt.float32
BF16 = mybir.dt.bfloat16
FP8 = mybir.dt.float8e4
DR = mybir.MatmulPerfMode.DoubleRow
AF = mybir.ActivationFunctionType

N_CORES = 8
B, S, E, A = 32, 1000, 1024, 512


def _even_chunks(start, total, maxc):
    """Split [start, start+total) into ceil(total/maxc) near-even chunks,
    each of even size (required by DoubleRow moving dim)."""
    assert total % 2 == 0
    n = max(1, -(-total // maxc))
    bounds = [start + ((i * total) // n) // 2 * 2 for i in range(n)]
    bounds.append(start + total)
    return [(bounds[i], bounds[i + 1] - bounds[i]) for i in range(n)]


def build(b_pc, s, e, a, reps=1, warm_weights=False):
    # warm_weights: skip the DRAM weight loads (timing-only simulation; the
    # measured rep-loop slope never sees the weight-load transient).
    assert e % P == 0 and a % P == 0
    n_s = -(-s // P)
    n_e = e // P
    n_a = a // P
    assert n_s == 8 and n_e == 8 and n_a == 4
    inv_den = float(s) ** -0.5
    s_tiles = [(t * P, min(P, s - t * P)) for t in range(n_s)]
    s_pad = n_s * P  # 1024; cols s..s_pad are zeros (zero-tail bf16 tile)
    h = a // 2  # PV column split: [0,h) and [h, a+2)
    nt_h = n_s // 2  # s-tiles per half

    nc = bacc.Bacc("TRN2")
    x = nc.dram_tensor("x", [b_pc, s, e], F32, kind="ExternalInput").ap()
    w_dram = {
        "q": nc.dram_tensor("Wq", [e, a], F32, kind="ExternalInput").ap(),
        "k": nc.dram_tensor("Wk", [e, a], F32, kind="ExternalInput").ap(),
        "v": nc.dram_tensor("Wv", [e, a], F32, kind="ExternalInput").ap(),
    }
    b_dram = {
        "q": nc.dram_tensor("bq", [a], F32, kind="ExternalInput").ap(),
        "k": nc.dram_tensor("bk", [a], F32, kind="ExternalInput").ap(),
        "v": nc.dram_tensor("bv", [a], F32, kind="ExternalInput").ap(),
    }
    out = nc.dram_tensor("out", [b_pc, s, a], BF16, kind="ExternalOutput").ap()

    with tile.TileContext(nc) as tc, ExitStack() as ctx:
        pool = ctx.enter_context(tc.tile_pool(name="sb", bufs=1))
        pp_proj = ctx.enter_context(tc.tile_pool(name="pp_proj", bufs=2, space="PSUM"))
        pp_score = ctx.enter_context(tc.tile_pool(name="pp_sc", bufs=2, space="PSUM"))
        pp_o1 = ctx.enter_context(tc.tile_pool(name="pp_o1", bufs=2, space="PSUM"))
        pp_o2 = ctx.enter_context(tc.tile_pool(name="pp_o2", bufs=2, space="PSUM"))

        # ---------------- weights / biases ----------------
        # w_sb (bf16): feeds the bf16 paths (q/k diagonal-block projections
        # and the v projection of s-tile 0). w8 (fp8, scaled by 32 to clear
        # the e4m3 subnormal range): DoubleRow pair tiles [P, 2, a] with
        # e-tiles (2*u2, 2*u2+1) on the pair dim, feeding the fp8 q/k/v
        # projections; the 1/32 rides the PSUM->SBUF activation scale.
        w_sb = {}
        w8 = {}
        warm_engs = [nc.vector, nc.gpsimd]
        for wi, nm in enumerate(("q", "k", "v")):
            tiles = []
            tiles8 = [pool.tile([P, 2, a], FP8, tag=f"w8_{nm}{u2}", bufs=1,
                                name=f"w8_{nm}{u2}")
                      for u2 in range(n_e // 2)]
            for u in range(n_e):
                w_r = pool.tile([P, a], BF16, tag=f"w_{nm}", bufs=n_e)
                if warm_weights:
                    # timing-only simulation: weights counted as resident
                    # (memsets spread across engines so no single engine's
                    # stream delays batch 0; first-exec-only transient anyway)
                    warm_engs[(wi * n_e + u) % 2].memset(w_r, 0.0)
                    if u % 2 == 0:
                        warm_engs[u % 2].memset(tiles8[u // 2], 0.0)
                    tiles.append(w_r)
                    continue
                w_stage = pool.tile([P, a], F32, tag="w_stage", bufs=2)
                nc.gpsimd.dma_start(
                    w_stage[:], w_dram[nm][u * P:(u + 1) * P, :])
                nc.vector.tensor_copy(w_r[:], w_stage[:])
                if nm == "v":
                    # unscaled: v's fp8 subnormal W error averages out in PV
                    nc.vector.tensor_copy(
                        tiles8[u // 2][:, u % 2, :], w_stage[:])
                else:
                    nc.vector.tensor_scalar_mul(
                        tiles8[u // 2][:, u % 2, :], w_stage[:], 32.0)
                tiles.append(w_r)
            w_sb[nm] = tiles
            w8[nm] = tiles8

        bias_sb = {}
        for nm in ("q", "k"):
            b_st = pool.tile([P, n_a], F32, tag=f"b_{nm}", bufs=1)
            nc.gpsimd.dma_start(
                b_st[:], b_dram[nm].rearrange("(m p) -> p m", p=P)
            )
            bias_sb[nm] = b_st
        bv_stage = pool.tile([1, a], F32)
        nc.gpsimd.dma_start(bv_stage[:], b_dram["v"][:])
        bv_bc = pool.tile([P, a], F32)
        nc.gpsimd.partition_broadcast(bv_bc[:], bv_stage[:])


        # additive causal mask for the diagonal block:
        # keep 0 where col q >= row k, else fill -1e9
        amask = pool.tile([P, P], F32)
        nc.gpsimd.memset(amask, 0.0)
        nc.gpsimd.affine_select(
            out=amask, in_=amask,
            compare_op=mybir.AluOpType.is_ge,
            fill=-1.0e9, base=0,
            pattern=[[1, P]], channel_multiplier=-1,
        )



        # ---------------- persistent per-tile SBUF slots ----------------
        x_slots = [pool.tile([P, e], F32, tag=f"x{t}", bufs=1, name=f"x{t}")
                   for t in range(n_s)]
        xbf = [pool.tile([P, e], BF16, tag=f"xbf{t}", bufs=1, name=f"xbf{t}")
               for t in range(n_s)]
        # zero the last tile's invalid rows once: transposes always read the
        # full 128 rows, so xT cols s..s_pad become clean zeros.
        if s_tiles[-1][1] < P:
            nc.gpsimd.memset(xbf[-1][:], 0.0)
        # xT[par][hi]: s-tile-major [P, nt_h, n_e, P]; par = batch parity.
        # xT8: fp8 copy feeding the DoubleRow projections (e-tile pairs
        # (2*u2, 2*u2+1) are adjacent on the u axis, so [:, t, 2u2:2u2+2, :]
        # is already a valid DR pair slice with a 256B pair stride).
        xT = [
            [pool.tile([P, nt_h, n_e, P], BF16, tag=f"xT{par}_{hi}", bufs=1,
                        name=f"xT{par}_{hi}")
             for hi in range(2)]
            for par in range(2)
        ]
        # xT8 layout is u-major [P, u2, 2, nt_h*P]: per (u2) the DR moving
        # slice [:, u2, :, :] spans the half's s-columns contiguously, so a
        # projection m-chunk is ONE accumulation chain over u2 on a full
        # PSUM bank (HW zeroes the whole bank at chain start, so per-region
        # sub-chains in one bank are illegal)
        xT8 = [
            [pool.tile([P, n_e // 2, 2, nt_h * P], FP8,
                       tag=f"xT8{par}_{hi}", bufs=1, name=f"xT8{par}_{hi}")
             for hi in range(2)]
            for par in range(2)
        ]
        # v as fp8 DoubleRow pair tiles [P, 2, a+2] (k-tiles 2p2, 2p2+1 on
        # the pair dim) with two constant ones-columns (softmax row-sums via
        # the PV matmul); double-buffered by batch parity: v(b+1) is computed
        # during C/D(b), which still reads v(b). Zeroed once so the unwritten
        # tail rows of the last k-tile contribute exact zeros.
        n_p2 = n_s // 2
        va_pad = -(-(a + 2) // 16) * 16  # DR pair-dim byte step must be %16
        vP = [
            [pool.tile([P, 2, va_pad], FP8, tag=f"vP{par}_{p2}", bufs=1,
                       name=f"vP{par}_{p2}")
             for p2 in range(n_p2)]
            for par in range(2)
        ]
        # pv-tile 0 (output rows 0..127) runs in bf16: its softmax support is
        # tiny (row r averages r+1 values), so fp8 v/exp rounding would land
        # nearly unaveraged in the output
        v0_bf = [pool.tile([P, a + 2], BF16, tag=f"v0bf{par}", bufs=1,
                           name=f"v0bf{par}")
                 for par in range(2)]
        for par in range(2):
            for p2 in range(n_p2):
                nc.gpsimd.memset(vP[par][p2][:], 0.0)
                nc.gpsimd.memset(vP[par][p2][:, :, a:a + 2], 1.0)
            nc.gpsimd.memset(v0_bf[par][:, a:a + 2], 1.0)
        # qT/kT fp8 DoubleRow pair tiles: a-tiles (2m, 2m+1) on the pair dim
        qkT = {
            nm: [pool.tile([P, 2, s_pad], FP8, tag=f"{nm}P{m2}", bufs=1,
                           name=f"{nm}P{m2}")
                 for m2 in range(n_a // 2)]
            for nm in ("q", "k")
        }
        # expT as fp8 DoubleRow pair tiles [P, 2, s - 256*p2]; pair-dim j
        # holds k-tiles (2*p2, 2*p2+1), columns are q - 256*p2. Zeroed once:
        # j=1's first 128 columns (the causally-masked sub-diagonal block of
        # tile 2*p2+1) and its unwritten tail rows stay exact zeros, which
        # lets every pv accumulate whole pairs including the diagonal one.
        expP = [pool.tile([P, 2, -(-(s - 2 * P * p2) // 16) * 16], FP8,
                          tag=f"expP{p2}", bufs=1, name=f"expP{p2}")
                for p2 in range(n_p2)]
        for p2 in range(n_p2):
            nc.gpsimd.memset(expP[p2][:, 1, :], 0.0)
        # bf16 copy of tile 0's diagonal exp block for the bf16 pv(0)
        et0_bf = pool.tile([P, P], BF16, tag="et0bf", bufs=1, name="et0_bf")
        # bf16-projected q/k for s-tile 0 ([P(a-part), m, P]): output rows
        # q < 128 average few softmax terms, so their scores bypass fp8
        # entirely (this also removes the old fp8 q/k rounding error there)
        qT0_bf = pool.tile([P, n_a, P], BF16, tag="qT0bf", bufs=1,
                           name="qT0_bf")
        kT0_bf = pool.tile([P, n_a, P], BF16, tag="kT0bf", bufs=1,
                           name="kT0_bf")

        # ---------------- emission helpers ----------------
        def emit_load(b, t):
            s0, sl = s_tiles[t]
            wsp = e // 2
            for qi in range(2):
                nc.sync.dma_start(
                    x_slots[t][:sl, qi * wsp:(qi + 1) * wsp],
                    x[b, s0:s0 + sl, qi * wsp:(qi + 1) * wsp],
                )

        def emit_conv_xbar(t, eng=None):
            sl = s_tiles[t][1]
            (eng or nc.gpsimd).tensor_copy(xbf[t][:sl, :], x_slots[t][:sl, :])
            return t

        def emit_xbar(b, t):
            par = b % 2
            hi = t // nt_h
            nc.sync.dma_start_transpose(
                xT[par][hi][:, t % nt_h, :, :], xbf[t][:])
            if t % nt_h == nt_h - 1:
                # half complete: emit the fp8 u-major repack for the
                # DoubleRow projections (one 4D copy per e-tile pair)
                for u2 in range(n_e // 2):
                    nc.vector.tensor_copy(
                        xT8[par][hi][:, u2, :, :].rearrange(
                            "p j (t c) -> p j t c", c=P),
                        xT[par][hi][:, :, 2 * u2:2 * u2 + 2, :].rearrange(
                            "p t j c -> p j t c"),
                    )

        def proj_chunk(b, nm, hi):
            # fp8 DoubleRow projection: 4 matmuls of 256-contraction per
            # (m, s-tile); per u2 the stationary W pair is loaded once and
            # streams the half's 4 s-tiles
            par = b % 2
            c0 = hi * (nt_h * P)
            cl_h = min(nt_h * P, s - c0)  # trim the s..s_pad padding columns
            # qT cols < 128 are only consumed by the bf16 diagonal path
            lo = P if (nm == "q" and hi == 0) else 0
            for m in range(n_a):
                mm = pp_proj.tile([P, 512], F32, tag="proj")
                for u2 in range(n_e // 2):
                    nc.tensor.matmul(
                        mm[:, lo:cl_h],
                        w8[nm][u2][:, :, m * P:(m + 1) * P],
                        xT8[par][hi][:, u2, :, lo:cl_h],
                        start=(u2 == 0), stop=(u2 == n_e // 2 - 1),
                        perf_mode=DR,
                    )
                nc.scalar.activation(
                    qkT[nm][m // 2][:, m % 2, c0 + lo:c0 + cl_h],
                    mm[:, lo:cl_h],
                    AF.Identity, bias=bias_sb[nm][:, m:m + 1],
                    scale=1.0 / 32.0,
                )

        def proj_diag(b):
            # bf16 q/k projections of s-tile 0 only, for the diagonal block
            par = b % 2
            for nm, dst in (("q", qT0_bf), ("k", kT0_bf)):
                for m in range(n_a):
                    mm = pp_score.tile([P, 512], F32, tag="score")
                    for u in range(n_e):
                        nc.tensor.matmul(
                            mm[:, :P],
                            w_sb[nm][u][:, m * P:(m + 1) * P],
                            xT[par][0][:, 0, u, :],
                            start=(u == 0), stop=(u == n_e - 1),
                        )
                    nc.scalar.activation(
                        dst[:, m, :], mm[:, :P],
                        AF.Identity, bias=bias_sb[nm][:, m:m + 1],
                    )

        def v_tile(b, t):
            par = b % 2
            s0, sl = s_tiles[t]
            vm = pp_proj.tile([P, 512], F32, tag="proj")
            if t == 0:
                # bf16 projection: rows < 128 read v (almost) unaveraged
                for u in range(n_e):
                    nc.tensor.matmul(
                        vm[:sl, :a],
                        xT[par][0][:, 0, u, :sl],
                        w_sb["v"][u][:],
                        start=(u == 0), stop=(u == n_e - 1),
                    )
                nc.vector.tensor_add(
                    vP[par][0][:sl, 0, :a], vm[:sl, :a], bv_bc[:sl, :])
                nc.vector.tensor_add(
                    v0_bf[par][:sl, :a], vm[:sl, :a], bv_bc[:sl, :])
                return
            # fp8 DoubleRow projection (unscaled Wv: the subnormal-range W
            # quantization error averages out across the softmax support)
            tl = t % nt_h
            for u2 in range(n_e // 2):
                nc.tensor.matmul(
                    vm[:sl, :a],
                    xT8[par][t // nt_h][:, u2, :, tl * P:tl * P + sl],
                    w8["v"][u2][:],
                    start=(u2 == 0), stop=(u2 == n_e // 2 - 1),
                    perf_mode=DR,
                )
            nc.vector.tensor_add(
                vP[par][t // 2][:sl, t % 2, :a], vm[:sl, :a], bv_bc[:sl, :])

        def scores_tile(t):
            k0, kl = s_tiles[t]
            ep = expP[t // 2]
            e0 = 2 * P * (t // 2)  # column origin of the pair tile
            # the diagonal block is its own small FIRST chunk: pv(t)'s last
            # accumulation pair needs exactly this chunk of expT, so its
            # mask->exp chain must resolve as early as possible
            chunks = [(k0, kl)]
            if k0 + kl < s:
                chunks += _even_chunks(k0 + kl, s - k0 - kl, 512)
            for pi, (c0, cl) in enumerate(chunks):
                sc = pp_score.tile([P, 512], F32, tag="score")
                if pi == 0 and t == 0:
                    # tile 0's diagonal block in bf16 (rows q < 128): feeds
                    # the bf16 pv(0) only; the fp8 pair tile's copy of this
                    # region is never read
                    for m in range(n_a):
                        nc.tensor.matmul(
                            sc[:kl, :kl],
                            kT0_bf[:, m, :], qT0_bf[:, m, :],
                            start=(m == 0), stop=(m == n_a - 1),
                        )
                    nc.vector.tensor_add(
                        sc[:kl, :kl], sc[:kl, :kl], amask[:kl, :kl]
                    )
                    nc.scalar.activation(
                        et0_bf[:kl, :kl], sc[:kl, :kl], AF.Exp,
                        scale=inv_den,
                    )
                    continue
                for m2 in range(n_a // 2):
                    nc.tensor.matmul(
                        sc[:kl, :cl],
                        qkT["k"][m2][:, :, k0:k0 + kl],
                        qkT["q"][m2][:, :, c0:c0 + cl],
                        start=(m2 == 0), stop=(m2 == n_a // 2 - 1),
                        perf_mode=DR,
                    )
                if pi == 0:
                    # diagonal block: additive causal mask in PSUM (DVE)
                    nc.vector.tensor_add(
                        sc[:kl, :kl], sc[:kl, :kl], amask[:kl, :kl]
                    )
                nc.scalar.activation(
                    ep[:kl, t % 2, c0 - e0:c0 - e0 + cl], sc[:kl, :cl],
                    AF.Exp, scale=inv_den,
                )

        def pv_tile(b, i):
            q0, il = s_tiles[i]
            op1 = pp_o1.tile([P, h], F32, tag="op1")
            op2 = pp_o2.tile([P, a - h + 2], F32, tag="op2")
            par = b % 2
            if i == 0:
                nc.tensor.matmul(op1[:il, :], et0_bf[:, 0:il],
                                 v0_bf[par][:, 0:h], start=True, stop=True)
                nc.tensor.matmul(op2[:il, :], et0_bf[:, 0:il],
                                 v0_bf[par][:, h:a + 2], start=True, stop=True)
            else:
                # fp8 DoubleRow over k-tile pairs; for even i the last pair's
                # j=1 slice (tile i+1) reads the once-zeroed sub-diagonal
                # columns, contributing exact zeros
                np2 = i // 2 + 1
                for p2 in range(np2):
                    c = q0 - 2 * P * p2
                    lhs = expP[p2][:, :, c:c + il]
                    nc.tensor.matmul(
                        op1[:il, :], lhs, vP[par][p2][:, :, 0:h],
                        start=(p2 == 0), stop=(p2 == np2 - 1),
                        perf_mode=DR,
                    )
                    nc.tensor.matmul(
                        op2[:il, :], lhs, vP[par][p2][:, :, h:a + 2],
                        start=(p2 == 0), stop=(p2 == np2 - 1),
                        perf_mode=DR,
                    )
            rec = pool.tile([P, 1], F32, tag="rec", bufs=4, name="rec")
            nc.vector.reciprocal(rec[:il, :], op2[:il, a - h:a - h + 1])
            # deep ring: out DMAs may lag several tiles behind the epilogue
            # (DMA_ENGINES serializes them behind prefetched stage-A traffic)
            o_sb = pool.tile([P, a], BF16, tag="o_sb", bufs=8, name="o_sb")
            # epilogue halves on different engines (ACT | DVE) so the
            # op1/op2 PSUM banks drain fast
            # both epilogue halves on DVE: ACT stays free for the exp
            # chain. outs ride the scalar HWDGE queue: everything there is
            # PE-gated, so a hoisted future-batch transpose on the sync
            # queue can never head-block the o_sb ring drain
            nc.vector.tensor_scalar_mul(
                o_sb[:il, 0:h], op1[:il, :], rec[:il, 0:1])
            nc.scalar.dma_start(out[b, q0:q0 + il, 0:h], o_sb[:il, 0:h])
            nc.vector.tensor_scalar_mul(
                o_sb[:il, h:a], op2[:il, 0:a - h], rec[:il, 0:1])
            nc.scalar.dma_start(out[b, q0:q0 + il, h:a], o_sb[:il, h:a])

        # ---------------- per-batch pipeline ----------------
        # prologue: cold stage A for batch 0 plus its v tiles (primes the
        # rotated loop body, which computes v(b+1) during C/D(b)). All loads
        # first (the sync queue pipelines them at full rate), then
        # convert+transpose per tile, converts alternating DVE/Pool.
        for t in range(n_s):
            emit_load(0, t)
        for t in range(n_s):
            emit_conv_xbar(t, nc.vector if t % 2 == 0 else nc.gpsimd)
            emit_xbar(0, t)
        for t in range(n_s):
            v_tile(0, t)

        rep_ctx = (tc.For_i(0, reps, 1, hint_engines=tuple(nc.engines),
                            staggered_reset=True)
                   if reps > 1 else None)
        if rep_ctx is not None:
            ctx.enter_context(rep_ctx)

        for b in range(b_pc):
            # scheduling tier: forbid the scheduler from hoisting batch b+1's
            # engine work (v-adds, bias copies, epilogues) into batch b's
            # streams — cross-batch hoists invert priorities on the in-order
            # engines. Work for batch b+1 emitted in section b (stage-A
            # prefetch) intentionally shares tier b.
            ctx_b = tc.tile_wait_until(b)
            ctx_b.__enter__()
            # every section preps the NEXT batch's x: (b+1)%b_pc — section
            # b_pc-1 preps batch 0 of the next rep-loop iteration, so the
            # loop back-edge barrier costs no refill (software pipelining
            # across the For_i back edge; the prologue primes iteration 1)
            for t in range(n_s):
                emit_load((b + 1) % b_pc, t)
            # ---- stage B (pure projections), with the next batch's
            # convert+transpose interleaved (xbf slots are free: batch b's
            # transposes ran during B(b-1); Pool/DMA are idle during B) ----
            for t in range(n_s):
                emit_conv_xbar(t)
                emit_xbar(b + 1, t)
            proj_diag(b)
            proj_chunk(b, "q", 0)
            proj_chunk(b, "k", 0)
            proj_chunk(b, "q", 1)
            proj_chunk(b, "k", 1)
            # ---- stage C/D with one tile of score lookahead, interleaved
            # with the NEXT batch's v tiles (their xT landed during B(b)):
            # ~1.7us of independent PE work per pv iteration covers the
            # cross-engine mask->exp->pv latency chains ----
            scores_tile(0)
            for i in range(n_s):
                if i + 1 < n_s:
                    scores_tile(i + 1)
                    v_tile(b + 1, i)
                    pv_tile(b, i)
                else:
                    # last iteration: pv first, then v — the v matmuls cover
                    # pv(7)'s epilogue/out drain so the next section's
                    # projections start against drained PSUM banks
                    pv_tile(b, i)
                    v_tile(b + 1, i)
            ctx_b.__exit__(None, None, None)

    nc.compile()
    return nc


_BUILT = {}


def _get_nc(b_pc, s, e, a):
    key = (b_pc, s, e, a)
    if key not in _BUILT:
        _BUILT[key] = build(b_pc, s, e, a)
    return _BUILT[key]


def run_sharded(inputs, b_pc, s, e, a, **run_kwargs):
    """Run the SPMD kernel over N_CORES cores, sharding batch dim of x."""
    x = np.ascontiguousarray(inputs["x"], dtype=np.float32)
    b_total = x.shape[0]
    assert b_total == b_pc * N_CORES
    shared = {
        "Wq": np.ascontiguousarray(inputs["Wq"], dtype=np.float32),
        "Wk": np.ascontiguousarray(inputs["Wk"], dtype=np.float32),
        "Wv": np.ascontiguousarray(inputs["Wv"], dtype=np.float32),
        "bq": np.ascontiguousarray(inputs["bq"], dtype=np.float32),
        "bk": np.ascontiguousarray(inputs["bk"], dtype=np.float32),
        "bv": np.ascontiguousarray(inputs["bv"], dtype=np.float32),
    }
    in_maps = [
        {"x": x[c * b_pc:(c + 1) * b_pc], **shared} for c in range(N_CORES)
    ]
    nc = _get_nc(b_pc, s, e, a)
    res = run_bass_kernel_spmd(nc, in_maps, core_ids=list(range(N_CORES)),
                               **run_kwargs)
    full = np.concatenate([res.results[c]["out"] for c in range(N_CORES)],
                          axis=0).astype(np.float32)
    return full, res


def kernel(**inputs) -> np.ndarray:
    out, _ = run_sharded(inputs, B // N_CORES, S, E, A)
    return out


# revision 71
# speedup vs baseline: 2.0231x; 1.0669x over previous
"""Masked self-attention Trainium2 Bass kernel.

Reference computation (per batch b):
    q = x @ Wq + bq ; k = x @ Wk + bk ; v = x @ Wv + bv      # [S, A]
    scores = (q @ k.T) / sqrt(S)  with causal mask            # [S, S]
    out = softmax(scores, axis=-1) @ v                        # [S, A]

Sharding: data-parallel over batch across 8 NeuronCores (B=32 -> 4 per core),
weights replicated. No collectives.

Per-core design, mixed precision (measured rel err ~5.2e-3 vs the 2e-2
gate). Everything DRAM-facing except the input is bf16/fp32; almost every
matmul runs in fp8e4 DoubleRow (2 MACs/cell/cycle), EXCEPT the paths feeding
output rows q < 128, whose softmax support is too small to average fp8
noise away:
  - q/k/v projections: fp8 DR over xT8 (fp8 copy of xT) and 32*W fp8 pair
    tiles (the 1/32 rides the PSUM->SBUF activation scale; Wv stays
    unscaled since its subnormal-range error averages out in PV).
  - scores: fp8 DR over qT/kT pair tiles; the diagonal block of s-tile 0
    (q,k < 128) instead uses dedicated bf16-projected qT0/kT0 (rows < 128
    thereby skip fp8 entirely, better than the all-bf16+fp8-rounding v1).
  - PV: fp8 DR over expT/v pair tiles [P, 2, *] pairing adjacent k-tiles;
    a once-zeroed sub-diagonal block in each pair tile's j=1 slice makes
    even-i accumulations read exact zeros for the causally-masked partner
    tile, so every pv(i) is ceil((i+1)/2) whole-pair DR matmuls. pv(0)
    runs in bf16 via et0_bf/v0_bf. Two constant ones-columns in v emit the
    softmax row-sums; DVE reciprocal scales both output halves; outputs
    store bf16 (host upcasts to fp32).

The PE never transposes: x [S,E] fp32 is DMA'd to SBUF, converted
fp32->bf16 (Pool/DVE), and transposed bf16 SBUF->SBUF by the XBAR
DMA-transpose unit (InstDmaTransposeAnt, out[p,u,c] = in[c, u*128+p])
straight into the s-tile-major xT layout [P, 4, n_e, P] (contiguous
destination per transpose, an XBAR requirement). xT8 is repacked u-major
[P, u2, 2, 512] by DVE 4D copies so each projection m-chunk is ONE
accumulation chain on a full PSUM bank (HW zeroes the whole bank at chain
start, so per-region sub-chains in one bank are illegal AND silently wrong).

Schedule (per section b of the software pipeline; tc.tile_wait_until(b)
tiers stop the scheduler from hoisting cross-batch work into earlier
in-order engine streams):
  - loads for batch b+1 (sync queue, 2 chunks per s-tile), then per s-tile
    convert (Pool/DVE) -> XBAR transpose (sync queue) -> fp8 repack (DVE,
    per half) for batch b+1.
  - stage B: bf16 diag projections of s-tile 0, then fp8 DR q/k
    projections (q half 0 skips cols < 128, served by the diag path).
  - stage C/D: scores per k-tile in causal-trimmed chunks, the diagonal
    block split out as its own small first chunk (pv(t)'s critical exp
    arrives early); additive -1e9 mask on DVE in PSUM; exp on ACT -> fp8
    pair tiles. One tile of score lookahead; batch b+1's v tiles (fp8 DR,
    tile 0 bf16) interleave between pv's to cover cross-engine latency;
    pv(7) runs before v(b+1,7) so the v matmuls cover its epilogue drain.
  - section b_pc-1 preps batch 0 of the NEXT For_i iteration, so the
    rep-loop back edge costs no refill (the prologue primes iteration 1).
o_sb rides an 8-deep ring (out DMAs may lag behind prefetched stage-A
DMA traffic on the serial DMA engines without stalling the epilogue).
"""

import numpy as np
from contextlib import ExitStack

import concourse.bass as bass
import concourse.mybir as mybir
import concourse.tile as tile
from concourse import bacc
from concourse.bass_utils import run_bass_kernel_spmd

P = 128
F32 = mybir.dt.float32
BF16 = mybir.dt.bfloat16
FP8 = mybir.dt.float8e4
DR = mybir.MatmulPerfMode.DoubleRow
AF = mybir.ActivationFunctionType

N_CORES = 8
B, S, E, A = 32, 1000, 1024, 512


def _even_chunks(start, total, maxc):
    """Split [start, start+total) into ceil(total/maxc) near-even chunks,
    each of even size (required by DoubleRow moving dim)."""
    assert total % 2 == 0
    n = max(1, -(-total // maxc))
    bounds = [start + ((i * total) // n) // 2 * 2 for i in range(n)]
    bounds.append(start + total)
    return [(bounds[i], bounds[i + 1] - bounds[i]) for i in range(n)]


def build(b_pc, s, e, a, reps=1, warm_weights=False):
    # warm_weights: skip the DRAM weight loads (timing-only simulation; the
    # measured rep-loop slope never sees the weight-load transient).
    assert e % P == 0 and a % P == 0
    n_s = -(-s // P)
    n_e = e // P
    n_a = a // P
    assert n_s == 8 and n_e == 8 and n_a == 4
    inv_den = float(s) ** -0.5
    s_tiles = [(t * P, min(P, s - t * P)) for t in range(n_s)]
    s_pad = n_s * P  # 1024; cols s..s_pad are zeros (zero-tail bf16 tile)
    h = a // 2  # PV column split: [0,h) and [h, a+2)
    nt_h = n_s // 2  # s-tiles per half

    nc = bacc.Bacc("TRN2")
    x = nc.dram_tensor("x", [b_pc, s, e], F32, kind="ExternalInput").ap()
    w_dram = {
        "q": nc.dram_tensor("Wq", [e, a], F32, kind="ExternalInput").ap(),
        "k": nc.dram_tensor("Wk", [e, a], F32, kind="ExternalInput").ap(),
        "v": nc.dram_tensor("Wv", [e, a], F32, kind="ExternalInput").ap(),
    }
    b_dram = {
        "q": nc.dram_tensor("bq", [a], F32, kind="ExternalInput").ap(),
        "k": nc.dram_tensor("bk", [a], F32, kind="ExternalInput").ap(),
        "v": nc.dram_tensor("bv", [a], F32, kind="ExternalInput").ap(),
    }
    out = nc.dram_tensor("out", [b_pc, s, a], BF16, kind="ExternalOutput").ap()

    with tile.TileContext(nc) as tc, ExitStack() as ctx:
        pool = ctx.enter_context(tc.tile_pool(name="sb", bufs=1))
        pp_proj = ctx.enter_context(tc.tile_pool(name="pp_proj", bufs=2, space="PSUM"))
        pp_score = ctx.enter_context(tc.tile_pool(name="pp_sc", bufs=2, space="PSUM"))
        pp_o1 = ctx.enter_context(tc.tile_pool(name="pp_o1", bufs=2, space="PSUM"))
        pp_o2 = ctx.enter_context(tc.tile_pool(name="pp_o2", bufs=2, space="PSUM"))

        # ---------------- weights / biases ----------------
        # w_sb (bf16): feeds the bf16 paths (q/k diagonal-block projections
        # and the v projection of s-tile 0). w8 (fp8, scaled by 32 to clear
        # the e4m3 subnormal range): DoubleRow pair tiles [P, 2, a] with
        # e-tiles (2*u2, 2*u2+1) on the pair dim, feeding the fp8 q/k/v
        # projections; the 1/32 rides the PSUM->SBUF activation scale.
        w_sb = {}
        w8 = {}
        warm_engs = [nc.vector, nc.gpsimd]
        for wi, nm in enumerate(("q", "k", "v")):
            tiles = []
            tiles8 = [pool.tile([P, 2, a], FP8, tag=f"w8_{nm}{u2}", bufs=1,
                                name=f"w8_{nm}{u2}")
                      for u2 in range(n_e // 2)]
            for u in range(n_e):
                w_r = pool.tile([P, a], BF16, tag=f"w_{nm}", bufs=n_e)
                if warm_weights:
                    # timing-only simulation: weights counted as resident
                    # (memsets spread across engines so no single engine's
                    # stream delays batch 0; first-exec-only transient anyway)
                    warm_engs[(wi * n_e + u) % 2].memset(w_r, 0.0)
                    if u % 2 == 0:
                        warm_engs[u % 2].memset(tiles8[u // 2], 0.0)
                    tiles.append(w_r)
                    continue
                w_stage = pool.tile([P, a], F32, tag="w_stage", bufs=2)
                nc.gpsimd.dma_start(
                    w_stage[:], w_dram[nm][u * P:(u + 1) * P, :])
                nc.vector.tensor_copy(w_r[:], w_stage[:])
                if nm == "v":
                    # unscaled: v's fp8 subnormal W error averages out in PV
                    nc.vector.tensor_copy(
                        tiles8[u // 2][:, u % 2, :], w_stage[:])
                else:
                    nc.vector.tensor_scalar_mul(
                        tiles8[u // 2][:, u % 2, :], w_stage[:], 32.0)
                tiles.append(w_r)
            w_sb[nm] = tiles
            w8[nm] = tiles8

        bias_sb = {}
        for nm in ("q", "k"):
            b_st = pool.tile([P, n_a], F32, tag=f"b_{nm}", bufs=1)
            nc.gpsimd.dma_start(
                b_st[:], b_dram[nm].rearrange("(m p) -> p m", p=P)
            )
            bias_sb[nm] = b_st
        bv_stage = pool.tile([1, a], F32)
        nc.gpsimd.dma_start(bv_stage[:], b_dram["v"][:])
        bv_bc = pool.tile([P, a], F32)
        nc.gpsimd.partition_broadcast(bv_bc[:], bv_stage[:])


        # additive causal mask for the diagonal block:
        # keep 0 where col q >= row k, else fill -1e9
        amask = pool.tile([P, P], F32)
        nc.gpsimd.memset(amask, 0.0)
        nc.gpsimd.affine_select(
            out=amask, in_=amask,
            compare_op=mybir.AluOpType.is_ge,
            fill=-1.0e9, base=0,
            pattern=[[1, P]], channel_multiplier=-1,
        )



        # ---------------- persistent per-tile SBUF slots ----------------
        x_slots = [pool.tile([P, e], F32, tag=f"x{t}", bufs=1, name=f"x{t}")
                   for t in range(n_s)]
        xbf = [pool.tile([P, e], BF16, tag=f"xbf{t}", bufs=1, name=f"xbf{t}")
               for t in range(n_s)]
        # zero the last tile's invalid rows once: transposes always read the
        # full 128 rows, so xT cols s..s_pad become clean zeros.
        if s_tiles[-1][1] < P:
            nc.gpsimd.memset(xbf[-1][:], 0.0)
        # xT[par][hi]: s-tile-major [P, nt_h, n_e, P]; par = batch parity.
        # xT8: fp8 copy feeding the DoubleRow projections (e-tile pairs
        # (2*u2, 2*u2+1) are adjacent on the u axis, so [:, t, 2u2:2u2+2, :]
        # is already a valid DR pair slice with a 256B pair stride).
        xT = [
            [pool.tile([P, nt_h, n_e, P], BF16, tag=f"xT{par}_{hi}", bufs=1,
                        name=f"xT{par}_{hi}")
             for hi in range(2)]
            for par in range(2)
        ]
        # xT8 layout is u-major [P, u2, 2, nt_h*P]: per (u2) the DR moving
        # slice [:, u2, :, :] spans the half's s-columns contiguously, so a
        # projection m-chunk is ONE accumulation chain over u2 on a full
        # PSUM bank (HW zeroes the whole bank at chain start, so per-region
        # sub-chains in one bank are illegal)
        xT8 = [
            [pool.tile([P, n_e // 2, 2, nt_h * P], FP8,
                       tag=f"xT8{par}_{hi}", bufs=1, name=f"xT8{par}_{hi}")
             for hi in range(2)]
            for par in range(2)
        ]
        # v as fp8 DoubleRow pair tiles [P, 2, a+2] (k-tiles 2p2, 2p2+1 on
        # the pair dim) with two constant ones-columns (softmax row-sums via
        # the PV matmul); double-buffered by batch parity: v(b+1) is computed
        # during C/D(b), which still reads v(b). Zeroed once so the unwritten
        # tail rows of the last k-tile contribute exact zeros.
        n_p2 = n_s // 2
        va_pad = -(-(a + 2) // 16) * 16  # DR pair-dim byte step must be %16
        vP = [
            [pool.tile([P, 2, va_pad], FP8, tag=f"vP{par}_{p2}", bufs=1,
                       name=f"vP{par}_{p2}")
             for p2 in range(n_p2)]
            for par in range(2)
        ]
        # pv-tile 0 (output rows 0..127) runs in bf16: its softmax support is
        # tiny (row r averages r+1 values), so fp8 v/exp rounding would land
        # nearly unaveraged in the output
        v0_bf = [pool.tile([P, a + 2], BF16, tag=f"v0bf{par}", bufs=1,
                           name=f"v0bf{par}")
                 for par in range(2)]
        for par in range(2):
            for p2 in range(n_p2):
                nc.gpsimd.memset(vP[par][p2][:], 0.0)
                nc.gpsimd.memset(vP[par][p2][:, :, a:a + 2], 1.0)
            nc.gpsimd.memset(v0_bf[par][:, a:a + 2], 1.0)
        # qT/kT fp8 DoubleRow pair tiles: a-tiles (2m, 2m+1) on the pair dim
        qkT = {
            nm: [pool.tile([P, 2, s_pad], FP8, tag=f"{nm}P{m2}", bufs=1,
                           name=f"{nm}P{m2}")
                 for m2 in range(n_a // 2)]
            for nm in ("q", "k")
        }
        # expT as fp8 DoubleRow pair tiles [P, 2, s - 256*p2]; pair-dim j
        # holds k-tiles (2*p2, 2*p2+1), columns are q - 256*p2. Zeroed once:
        # j=1's first 128 columns (the causally-masked sub-diagonal block of
        # tile 2*p2+1) and its unwritten tail rows stay exact zeros, which
        # lets every pv accumulate whole pairs including the diagonal one.
        expP = [pool.tile([P, 2, -(-(s - 2 * P * p2) // 16) * 16], FP8,
                          tag=f"expP{p2}", bufs=1, name=f"expP{p2}")
                for p2 in range(n_p2)]
        for p2 in range(n_p2):
            nc.gpsimd.memset(expP[p2][:, 1, :], 0.0)
        # bf16 copy of tile 0's diagonal exp block for the bf16 pv(0)
        et0_bf = pool.tile([P, P], BF16, tag="et0bf", bufs=1, name="et0_bf")
        # bf16-projected q/k for s-tile 0 ([P(a-part), m, P]): output rows
        # q < 128 average few softmax terms, so their scores bypass fp8
        # entirely (this also removes the old fp8 q/k rounding error there)
        qT0_bf = pool.tile([P, n_a, P], BF16, tag="qT0bf", bufs=1,
                           name="qT0_bf")
        kT0_bf = pool.tile([P, n_a, P], BF16, tag="kT0bf", bufs=1,
                           name="kT0_bf")

        # ---------------- emission helpers ----------------
        def emit_load(b, t):
            s0, sl = s_tiles[t]
            wsp = e // 2
            for qi in range(2):
                nc.sync.dma_start(
                    x_slots[t][:sl, qi * wsp:(qi + 1) * wsp],
                    x[b, s0:s0 + sl, qi * wsp:(qi + 1) * wsp],
                )

        def emit_conv_xbar(t, eng=None):
            sl = s_tiles[t][1]
            (eng or nc.gpsimd).tensor_copy(xbf[t][:sl, :], x_slots[t][:sl, :])
            return t

        def emit_xbar(b, t):
            par = b % 2
            hi = t // nt_h
            nc.sync.dma_start_transpose(
                xT[par][hi][:, t % nt_h, :, :], xbf[t][:])
            if t % nt_h == nt_h - 1:
                # half complete: emit the fp8 u-major repack for the
                # DoubleRow projections (one 4D copy per e-tile pair)
                for u2 in range(n_e // 2):
                    nc.vector.tensor_copy(
                        xT8[par][hi][:, u2, :, :].rearrange(
                            "p j (t c) -> p j t c", c=P),
                        xT[par][hi][:, :, 2 * u2:2 * u2 + 2, :].rearrange(
                            "p t j c -> p j t c"),
                    )

        def proj_chunk(b, nm, hi):
            # fp8 DoubleRow projection: 4 matmuls of 256-contraction per
            # (m, s-tile); per u2 the stationary W pair is loaded once and
            # streams the half's 4 s-tiles
            par = b % 2
            c0 = hi * (nt_h * P)
            cl_h = min(nt_h * P, s - c0)  # trim the s..s_pad padding columns
            # qT cols < 128 are only consumed by the bf16 diagonal path
            lo = P if (nm == "q" and hi == 0) else 0
            for m in range(n_a):
                mm = pp_proj.tile([P, 512], F32, tag="proj")
                for u2 in range(n_e // 2):
                    nc.tensor.matmul(
                        mm[:, lo:cl_h],
                        w8[nm][u2][:, :, m * P:(m + 1) * P],
                        xT8[par][hi][:, u2, :, lo:cl_h],
                        start=(u2 == 0), stop=(u2 == n_e // 2 - 1),
                        perf_mode=DR,
                    )
                nc.scalar.activation(
                    qkT[nm][m // 2][:, m % 2, c0 + lo:c0 + cl_h],
                    mm[:, lo:cl_h],
                    AF.Identity, bias=bias_sb[nm][:, m:m + 1],
                    scale=1.0 / 32.0,
                )

        def proj_diag(b):
            # bf16 q/k projections of s-tile 0 only, for the diagonal block
            par = b % 2
            for nm, dst in (("q", qT0_bf), ("k", kT0_bf)):
                for m in range(n_a):
                    mm = pp_score.tile([P, 512], F32, tag="score")
                    for u in range(n_e):
                        nc.tensor.matmul(
                            mm[:, :P],
                            w_sb[nm][u][:, m * P:(m + 1) * P],
                            xT[par][0][:, 0, u, :],
                            start=(u == 0), stop=(u == n_e - 1),
                        )
                    nc.scalar.activation(
                        dst[:, m, :], mm[:, :P],
                        AF.Identity, bias=bias_sb[nm][:, m:m + 1],
                    )

        def v_tile(b, t):
            par = b % 2
            s0, sl = s_tiles[t]
            vm = pp_proj.tile([P, 512], F32, tag="proj")
            if t == 0:
                # bf16 projection: rows < 128 read v (almost) unaveraged
                for u in range(n_e):
                    nc.tensor.matmul(
                        vm[:sl, :a],
                        xT[par][0][:, 0, u, :sl],
                        w_sb["v"][u][:],
                        start=(u == 0), stop=(u == n_e - 1),
                    )
                nc.vector.tensor_add(
                    vP[par][0][:sl, 0, :a], vm[:sl, :a], bv_bc[:sl, :])
                nc.vector.tensor_add(
                    v0_bf[par][:sl, :a], vm[:sl, :a], bv_bc[:sl, :])
                return
            # fp8 DoubleRow projection (unscaled Wv: the subnormal-range W
            # quantization error averages out across the softmax support)
            tl = t % nt_h
            for u2 in range(n_e // 2):
                nc.tensor.matmul(
                    vm[:sl, :a],
                    xT8[par][t // nt_h][:, u2, :, tl * P:tl * P + sl],
                    w8["v"][u2][:],
                    start=(u2 == 0), stop=(u2 == n_e // 2 - 1),
                    perf_mode=DR,
                )
            nc.vector.tensor_add(
                vP[par][t // 2][:sl, t % 2, :a], vm[:sl, :a], bv_bc[:sl, :])

        def scores_tile(t):
            k0, kl = s_tiles[t]
            ep = expP[t // 2]
            e0 = 2 * P * (t // 2)  # column origin of the pair tile
            # the diagonal block is its own small FIRST chunk: pv(t)'s last
            # accumulation pair needs exactly this chunk of expT, so its
            # mask->exp chain must resolve as early as possible
            chunks = [(k0, kl)]
            if k0 + kl < s:
                chunks += _even_chunks(k0 + kl, s - k0 - kl, 512)
            for pi, (c0, cl) in enumerate(chunks):
                sc = pp_score.tile([P, 512], F32, tag="score")
                if pi == 0 and t == 0:
                    # tile 0's diagonal block in bf16 (rows q < 128): feeds
                    # the bf16 pv(0) only; the fp8 pair tile's copy of this
                    # region is never read
                    for m in range(n_a):
                        nc.tensor.matmul(
                            sc[:kl, :kl],
                            kT0_bf[:, m, :], qT0_bf[:, m, :],
                            start=(m == 0), stop=(m == n_a - 1),
                        )
                    nc.vector.tensor_add(
                        sc[:kl, :kl], sc[:kl, :kl], amask[:kl, :kl]
                    )
                    nc.scalar.activation(
                        et0_bf[:kl, :kl], sc[:kl, :kl], AF.Exp,
                        scale=inv_den,
                    )
                    continue
                for m2 in range(n_a // 2):
                    nc.tensor.matmul(
                        sc[:kl, :cl],
                        qkT["k"][m2][:, :, k0:k0 + kl],
                        qkT["q"][m2][:, :, c0:c0 + cl],
                        start=(m2 == 0), stop=(m2 == n_a // 2 - 1),
                        perf_mode=DR,
                    )
                if pi == 0:
                    # diagonal block: additive causal mask in PSUM (DVE)
                    nc.vector.tensor_add(
                        sc[:kl, :kl], sc[:kl, :kl], amask[:kl, :kl]
                    )
                nc.scalar.activation(
                    ep[:kl, t % 2, c0 - e0:c0 - e0 + cl], sc[:kl, :cl],
                    AF.Exp, scale=inv_den,
                )

        def pv_tile(b, i):
            q0, il = s_tiles[i]
            op1 = pp_o1.tile([P, h], F32, tag="op1")
            op2 = pp_o2.tile([P, a - h + 2], F32, tag="op2")
            par = b % 2
            if i == 0:
                nc.tensor.matmul(op1[:il, :], et0_bf[:, 0:il],
                                 v0_bf[par][:, 0:h], start=True, stop=True)
                nc.tensor.matmul(op2[:il, :], et0_bf[:, 0:il],
                                 v0_bf[par][:, h:a + 2], start=True, stop=True)
            else:
                # fp8 DoubleRow over k-tile pairs; for even i the last pair's
                # j=1 slice (tile i+1) reads the once-zeroed sub-diagonal
                # columns, contributing exact zeros
                np2 = i // 2 + 1
                for p2 in range(np2):
                    c = q0 - 2 * P * p2
                    lhs = expP[p2][:, :, c:c + il]
                    nc.tensor.matmul(
                        op1[:il, :], lhs, vP[par][p2][:, :, 0:h],
                        start=(p2 == 0), stop=(p2 == np2 - 1),
                        perf_mode=DR,
                    )
                    nc.tensor.matmul(
                        op2[:il, :], lhs, vP[par][p2][:, :, h:a + 2],
                        start=(p2 == 0), stop=(p2 == np2 - 1),
                        perf_mode=DR,
                    )
            rec = pool.tile([P, 1], F32, tag="rec", bufs=4, name="rec")
            nc.vector.reciprocal(rec[:il, :], op2[:il, a - h:a - h + 1])
            # deep ring: out DMAs may lag several tiles behind the epilogue
            # (DMA_ENGINES serializes them behind prefetched stage-A traffic)
            o_sb = pool.tile([P, a], BF16, tag="o_sb", bufs=8, name="o_sb")
            # epilogue halves on different engines (ACT | DVE) so the
            # op1/op2 PSUM banks drain fast
            # both epilogue halves on DVE: ACT stays free for the exp
            # chain. outs ride the scalar HWDGE queue: everything there is
            # PE-gated, so a hoisted future-batch transpose on the sync
            # queue can never head-block the o_sb ring drain
            nc.vector.tensor_scalar_mul(
                o_sb[:il, 0:h], op1[:il, :], rec[:il, 0:1])
            nc.sync.dma_start(out[b, q0:q0 + il, 0:h], o_sb[:il, 0:h])
            nc.vector.tensor_scalar_mul(
                o_sb[:il, h:a], op2[:il, 0:a - h], rec[:il, 0:1])
            nc.sync.dma_start(out[b, q0:q0 + il, h:a], o_sb[:il, h:a])

        # ---------------- per-batch pipeline ----------------
        # prologue: cold stage A for batch 0 plus its v tiles (primes the
        # rotated loop body, which computes v(b+1) during C/D(b)). All loads
        # first (the sync queue pipelines them at full rate), then
        # convert+transpose per tile, converts alternating DVE/Pool.
        for t in range(n_s):
            emit_load(0, t)
        for t in range(n_s):
            emit_conv_xbar(t, nc.vector if t % 2 == 0 else nc.gpsimd)
            emit_xbar(0, t)
        for t in range(n_s):
            v_tile(0, t)

        rep_ctx = (tc.For_i(0, reps, 1, hint_engines=tuple(nc.engines),
                            staggered_reset=True)
                   if reps > 1 else None)
        if rep_ctx is not None:
            ctx.enter_context(rep_ctx)

        for b in range(b_pc):
            # scheduling tier: forbid the scheduler from hoisting batch b+1's
            # engine work (v-adds, bias copies, epilogues) into batch b's
            # streams — cross-batch hoists invert priorities on the in-order
            # engines. Work for batch b+1 emitted in section b (stage-A
            # prefetch) intentionally shares tier b.
            ctx_b = tc.tile_wait_until(b)
            ctx_b.__enter__()
            # every section preps the NEXT batch's x: (b+1)%b_pc — section
            # b_pc-1 preps batch 0 of the next rep-loop iteration, so the
            # loop back-edge barrier costs no refill (software pipelining
            # across the For_i back edge; the prologue primes iteration 1)
            for t in range(n_s):
                emit_load((b + 1) % b_pc, t)
            # ---- stage B (pure projections), with the next batch's
            # convert+transpose interleaved (xbf slots are free: batch b's
            # transposes ran during B(b-1); Pool/DMA are idle during B) ----
            for t in range(n_s):
                emit_conv_xbar(t, nc.vector if t % 2 == 0 else nc.gpsimd)
                emit_xbar(b + 1, t)
            proj_diag(b)
            proj_chunk(b, "q", 0)
            proj_chunk(b, "k", 0)
            proj_chunk(b, "q", 1)
            proj_chunk(b, "k", 1)
            # ---- stage C/D with one tile of score lookahead, interleaved
            # with the NEXT batch's v tiles (their xT landed during B(b)):
            # ~1.7us of independent PE work per pv iteration covers the
            # cross-engine mask->exp->pv latency chains ----
            scores_tile(0)
            for i in range(n_s):
                if i + 1 < n_s:
                    scores_tile(i + 1)
                    v_tile(b + 1, i)
                    pv_tile(b, i)
                else:
                    # last iteration: pv first, then v — the v matmuls cover
                    # pv(7)'s epilogue/out drain so the next section's
                    # projections start against drained PSUM banks
                    pv_tile(b, i)
                    v_tile(b + 1, i)
            ctx_b.__exit__(None, None, None)

    nc.compile()
    return nc


_BUILT = {}


def _get_nc(b_pc, s, e, a):
    key = (b_pc, s, e, a)
    if key not in _BUILT:
        _BUILT[key] = build(b_pc, s, e, a)
    return _BUILT[key]


def run_sharded(inputs, b_pc, s, e, a, **run_kwargs):
    """Run the SPMD kernel over N_CORES cores, sharding batch dim of x."""
    x = np.ascontiguousarray(inputs["x"], dtype=np.float32)
    b_total = x.shape[0]
    assert b_total == b_pc * N_CORES
    shared = {
        "Wq": np.ascontiguousarray(inputs["Wq"], dtype=np.float32),
        "Wk": np.ascontiguousarray(inputs["Wk"], dtype=np.float32),
        "Wv": np.ascontiguousarray(inputs["Wv"], dtype=np.float32),
        "bq": np.ascontiguousarray(inputs["bq"], dtype=np.float32),
        "bk": np.ascontiguousarray(inputs["bk"], dtype=np.float32),
        "bv": np.ascontiguousarray(inputs["bv"], dtype=np.float32),
    }
    in_maps = [
        {"x": x[c * b_pc:(c + 1) * b_pc], **shared} for c in range(N_CORES)
    ]
    nc = _get_nc(b_pc, s, e, a)
    res = run_bass_kernel_spmd(nc, in_maps, core_ids=list(range(N_CORES)),
                               **run_kwargs)
    full = np.concatenate([res.results[c]["out"] for c in range(N_CORES)],
                          axis=0).astype(np.float32)
    return full, res


def kernel(**inputs) -> np.ndarray:
    out, _ = run_sharded(inputs, B // N_CORES, S, E, A)
    return out
